# revision 9
# baseline (speedup 1.0000x reference)
"""CCPL contrastive loss kernel for Trainium2 (8 NeuronCores, SPMD data-parallel).

Contract: kernel(**inputs) takes FULL unsharded inputs, returns the FULL scalar
loss (float32, shape ()).  Host does indexing-only gathers (sid in [0,30) means
only the top-left 32x32 corner of every feature map is read); core b processes
batch b end-to-end on device; host sums the 8 partial CE sums / (8*512).

v2 redesign vs baseline (64.2us):
 - center duplicated 8x in the upload so x = neigh - center is a flat fp16
   SBUF sub (DVE 2x mode) instead of a stride-0-broadcast 1x op
 - f' = y * rn where rn = exp(-0.5 ln ssq - 0.5 ln tau): both branches carry
   1/sqrt(tau), so the Gram G' = fq'^T fk' is already G/tau
 - Gram computed TRANSPOSED (G'[t-block, s]) into one 4-bank PSUM tile; one
   merged exp per layer (ACT, bias -0.5/tau keeps E' in fp16 range);
   row-sums Z = ones^T E' as PE matmuls accumulating over the 4 t-blocks
 - sum_s ln Z via ACT Ln with accum_out (per-layer scalar, no DVE reduces)
 - sum_s l_pos via fp16 pprod + DVE tensor_scalar accum_out
 - l0-2: q/k band-stacked in partitions [0:Co]/[64:64+Co] for mm2/ssq/bc
   (selector matmuls), halving small-op count
 - engine balancing: relu split ACT/DVE, l3 subs on GpSimd, PSUM rings sized
   to exactly 8 banks (mm1 2 + sm 2 + g 4)
"""

import numpy as np
from contextlib import ExitStack

import concourse.bass as bass
import concourse.bacc as bacc
import concourse.tile as tile
from concourse import mybir
from concourse.bass_utils import run_bass_kernel_spmd

F32 = mybir.dt.float32
F16 = mybir.dt.float16
ALU = mybir.AluOpType
AF = mybir.ActivationFunctionType

# Force Exp/Ln/Relu into the one table set containing all three so the kernel
# pays a single ACT_TABLE_LOAD.
_COMBINED_SET = "natural_log_exp_and_others"
_orig_get_tables = bacc.get_activation_tables


def _patched_get_tables(arch):
    t = _orig_get_tables(arch)
    strip = {AF.Exp, AF.Ln, AF.Relu}
    return {
        name: (fns if name == _COMBINED_SET else (set(fns) - strip))
        for name, fns in t.items()
    }


bacc.get_activation_tables = _patched_get_tables

TAU = 0.07
NCORES = 8
S = 512
NS = 64
CS = [64, 128, 256, 512]
COUT = [16, 32, 64, 128]
KC = [1, 1, 2, 4]
NCH = sum(KC)
LORD = (1, 3, 2, 0)          # processing order (also x/wts blob order)
_DH = np.array([0, 0, 0, 1, 1, 2, 2, 2], dtype=np.int64)
_DW = np.array([0, 1, 2, 0, 2, 0, 1, 2], dtype=np.int64)
BIAS_RN = -0.5 * float(np.log(TAU))   # rn' = exp(-0.5 ln ssq + BIAS_RN)
BIAS_E = -0.5 / TAU                   # exp(G' - 0.5/tau)
COMB_C = 4 * S * 0.5 / TAU            # restores the BIAS_E shift in sum_s lnZ

# chunk indices in LORD order ------------------------------------------------
CH = {}
_c = 0
for _l in LORD:
    for _k in range(KC[_l]):
        CH[(_l, _k)] = _c
        _c += 1

# weight blob column offsets, LORD order so DMA slices are contiguous
W1C, W2C, WBLK = {}, {}, {}
_c = 0
for _l in LORD:
    _s = _c
    for _k in range(KC[_l]):
        W1C[(_l, _k)] = _c
        _c += CS[_l]
    for _k in range(KC[_l]):
        W2C[(_l, _k)] = _c
        _c += COUT[_l]
    WBLK[_l] = (_s, _c)
WTOT = _c

# aux blob (f32): cols 0..7 b1 chunks, 8..11 b2 (band-stacked for l<3), 12 ones
B1C = {}
_c = 0
for _l in range(4):
    for _m in range(KC[_l]):
        B1C[(_l, _m)] = _c
        _c += 1
B2C = {l: 8 + l for l in range(4)}
OCOL = 12
RNC = 13          # BIAS_RN column
BEC = 14          # BIAS_E column
AUXW = 16
# auxh (f16) [128, 8]: col 0 ones column; cols 1+2l / 2+2l: ssq band selectors
# auxh2 (f16) [2, 512]: cols 0:128 ones row (row 0); bsel_l at 128+128l
SEL2 = {l: 1 + 2 * l for l in range(3)}
BSEL = {l: 128 + 128 * l for l in range(3)}

# engine assignment knobs ----------------------------------------------------
# relu engine per (layer, m, branch): 'a' = ACT, 'v' = DVE
RELU_ENG = {}
for _l in range(4):
    for _m in range(KC[_l]):
        for _b in range(2):
            RELU_ENG[(_l, _m, _b)] = 'a' if (_m + _b) % 2 == 0 else 'v'
SUB_ENG = {0: 'v', 1: 'v', 2: 'v', 3: 'v'}   # all subs on DVE (GpSimd contends for the SBUF port)
EVICT_ENG = {0: 'v', 1: 'v', 2: 'v', 3: 'a'}


def _build_nc(relu_eng=None, sub_eng=None, evict_eng=None):
    relu_eng = relu_eng or RELU_ENG
    sub_eng = sub_eng or SUB_ENG
    evict_eng = evict_eng or EVICT_ENG
    nc = bacc.Bacc()
    xq = nc.dram_tensor("xq", [NCH * 128, 1024], F16, kind="ExternalInput")
    xk = nc.dram_tensor("xk", [NCH * 128, 1024], F16, kind="ExternalInput")
    wts = nc.dram_tensor("wts", [128, WTOT], F16, kind="ExternalInput")
    aux = nc.dram_tensor("aux", [128, AUXW], F32, kind="ExternalInput")
    auxh = nc.dram_tensor("auxh", [128, 8], F16, kind="ExternalInput")
    auxh2 = nc.dram_tensor("auxh2", [2, 512], F16, kind="ExternalInput")
    out = nc.dram_tensor("out", [1, 1], F32, kind="ExternalOutput")

    with ExitStack() as ctx:
        tc = ctx.enter_context(tile.TileContext(nc))
        const = ctx.enter_context(tc.tile_pool(name="const", bufs=1))
        work = ctx.enter_context(tc.tile_pool(name="work", bufs=2))
        fpool = ctx.enter_context(tc.tile_pool(name="fpool", bufs=2))
        epool = ctx.enter_context(tc.tile_pool(name="epool", bufs=2))
        # PSUM: mm1 (2x1 bank) + sm (2x1 bank) + g (1x4 banks) = 8 banks
        pmm = ctx.enter_context(tc.tile_pool(name="pmm", bufs=2, space="PSUM"))
        psm = ctx.enter_context(tc.tile_pool(name="psm", bufs=2, space="PSUM"))
        pg = ctx.enter_context(tc.tile_pool(name="pg", bufs=1, space="PSUM"))

        xq_s = const.tile([128, NCH, 1024], F16)
        xk_s = const.tile([128, NCH, 1024], F16)
        wall = const.tile([128, WTOT], F16)
        aall = const.tile([128, AUXW], F32)
        hall = const.tile([128, 8], F16)
        hrow = const.tile([2, 512], F16)
        xall = (xq_s, xk_s)

        # x DMAs on gpsimd (cheap issues), wts/aux on sync
        rq = xq.rearrange("(n p) m -> p n m", p=128)
        rk = xk.rearrange("(n p) m -> p n m", p=128)
        rearr = (rq, rk)
        for l in (1, 3):
            a0, a1 = CH[(l, 0)], CH[(l, 0)] + KC[l]
            for b in range(2):
                if l == 3:   # chunk pairs for pipelining
                    for c in range(a0, a1, 2):
                        nc.gpsimd.dma_start(out=xall[b][:, c:c + 2, :],
                                            in_=rearr[b][:, c:c + 2, :])
                else:
                    nc.gpsimd.dma_start(out=xall[b][:, a0:a1, :],
                                        in_=rearr[b][:, a0:a1, :])
        for l in (2, 0):
            a0, a1 = CH[(l, 0)], CH[(l, 0)] + KC[l]
            for b in range(2):
                nc.scalar.dma_start(out=xall[b][:, a0:a1, :],
                                    in_=rearr[b][:, a0:a1, :])
        w0, w1_ = WBLK[LORD[0]]
        nc.sync.dma_start(out=wall[:, w0:w1_], in_=wts[:, w0:w1_])
        nc.sync.dma_start(out=hall, in_=auxh[:, :])
        nc.sync.dma_start(out=hrow, in_=auxh2[:, :])
        nc.sync.dma_start(out=aall, in_=aux[:, :])
        for l in LORD[1:]:
            w0, w1_ = WBLK[l]
            nc.sync.dma_start(out=wall[:, w0:w1_], in_=wts[:, w0:w1_])

        # shared state
        xsub = [const.tile([128, NCH, S], F16, tag=f"xsub{b}",
                           name=f"xsub{b}") for b in range(2)]
        y2st = const.tile([128, S], F16)       # band-stacked y^2, pre-zeroed
        catb = const.tile([1, 8], F32)         # per-layer sum_s lnZ'
        catp = const.tile([128, 4], F32)       # per-layer banded sum_s l_pos'
        nc.gpsimd.memset(y2st[:, :], 0.0)
        nc.gpsimd.memset(catb[:, :], 0.0)
        nc.gpsimd.memset(catp[:, :], 0.0)

        st = {}   # per-layer tiles carried between phases

        def emit_subs(l):
            a0, a1 = CH[(l, 0)], CH[(l, 0)] + KC[l]
            eng = nc.gpsimd if sub_eng[l] == 'g' else nc.vector
            step = 2 if l == 3 else (a1 - a0)
            for b in range(2):
                for c in range(a0, a1, step):
                    eng.tensor_sub(
                        out=xsub[b][:, c:c + step, :],
                        in0=xall[b][:, c:c + step, 0:512],
                        in1=xall[b][:, c:c + step, 512:1024])

        def emit_mlp1(l):
            C, Co, K = CS[l], COUT[l], KC[l]
            h = const.tile([128, K, 2, S], F16, tag=f"h{l}")
            for b in range(2):
                for m in range(K):
                    rows = min(128, C - m * 128)
                    mm1 = pmm.tile([128, S], F32, tag="mm1")
                    for k in range(K):
                        c0 = W1C[(l, k)] + m * 128
                        nc.tensor.matmul(
                            mm1[0:rows, :],
                            lhsT=wall[:, c0:c0 + rows],
                            rhs=xsub[b][:, CH[(l, k)], :],
                            start=(k == 0), stop=(k == K - 1))
                    bc1 = B1C[(l, m)]
                    if relu_eng[(l, m, b)] == 'a':
                        nc.scalar.activation(
                            out=h[0:rows, m, b, :], in_=mm1[0:rows, :],
                            func=AF.Relu, bias=aall[0:rows, bc1:bc1 + 1],
                            scale=1.0)
                    else:
                        nc.vector.tensor_scalar(
                            out=h[0:rows, m, b, :], in0=mm1[0:rows, :],
                            scalar1=aall[0:rows, bc1:bc1 + 1], scalar2=0.0,
                            op0=ALU.add, op1=ALU.max)
            st[(l, 'h')] = h

        def emit_mlp2(l):
            C, Co, K = CS[l], COUT[l], KC[l]
            h = st[(l, 'h')]
            if l == 3:
                mm2 = pg.tile([128, 2, S], F32, tag="g")
                for b in range(2):
                    for k in range(K):
                        c0 = W2C[(l, k)]
                        nc.tensor.matmul(
                            mm2[:, b, :],
                            lhsT=wall[0:128, c0:c0 + Co],
                            rhs=h[:, k, b, :],
                            start=(k == 0), stop=(k == K - 1))
            else:
                mm2 = psm.tile([128, S], F32, tag="sm")
                for b in range(2):
                    for k in range(K):
                        rows = min(128, C - k * 128)
                        c0 = W2C[(l, k)]
                        nc.tensor.matmul(
                            mm2[64 * b:64 * b + Co, :],
                            lhsT=wall[0:rows, c0:c0 + Co],
                            rhs=h[0:rows, k, b, :],
                            start=(k == 0), stop=(k == K - 1))
            st[(l, 'mm2')] = mm2

        def _evict(l, outt, int_, bias_ap):
            if evict_eng[l] == 'a':
                nc.scalar.activation(out=outt, in_=int_, func=AF.Identity,
                                     bias=bias_ap, scale=1.0)
            else:
                nc.vector.tensor_scalar_add(out=outt, in0=int_,
                                            scalar1=bias_ap)

        def emit_tail1(l):
            C, Co, K = CS[l], COUT[l], KC[l]
            mm2 = st[(l, 'mm2')]
            b2c = B2C[l]
            if l == 3:
                yst = fpool.tile([128, 2, S], F16, tag="yst3")
                _evict(l, yst[:, :, :], mm2[:, :, :], aall[:, b2c:b2c + 1])
                y2q = work.tile([128, S], F16, tag="y2q")
                y2k = work.tile([128, S], F16, tag="y2k")
                nc.vector.tensor_mul(out=y2q, in0=yst[:, 0, :], in1=yst[:, 0, :])
                nc.vector.tensor_mul(out=y2k, in0=yst[:, 1, :], in1=yst[:, 1, :])
                ssq_q = psm.tile([1, S], F32, tag="sm")
                ssq_k = psm.tile([1, S], F32, tag="sm")
                nc.tensor.matmul(ssq_q, lhsT=hall[:, 0:1], rhs=y2q,
                                 start=True, stop=True)
                nc.tensor.matmul(ssq_k, lhsT=hall[:, 0:1], rhs=y2k,
                                 start=True, stop=True)
                t3q = work.tile([1, S], F32, tag="t3q")
                t3k = work.tile([1, S], F32, tag="t3k")
                nc.scalar.activation(out=t3q, in_=ssq_q, func=AF.Ln)
                nc.scalar.activation(out=t3k, in_=ssq_k, func=AF.Ln)
                rnq = work.tile([1, S], F16, tag="rnq")
                rnk = work.tile([1, S], F16, tag="rnk")
                nc.scalar.activation(out=rnq, in_=t3q, func=AF.Exp,
                                     scale=-0.5, bias=aall[0:1, RNC:RNC + 1])
                nc.scalar.activation(out=rnk, in_=t3k, func=AF.Exp,
                                     scale=-0.5, bias=aall[0:1, RNC:RNC + 1])
                bcq = psm.tile([128, S], F32, tag="sm")
                nc.tensor.matmul(bcq, lhsT=hrow[0:1, 0:128], rhs=rnq,
                                 start=True, stop=True)
                bck = psm.tile([128, S], F32, tag="sm")
                nc.tensor.matmul(bck, lhsT=hrow[0:1, 0:128], rhs=rnk,
                                 start=True, stop=True)
                fq = fpool.tile([128, S], F16, tag="fq")
                fk = fpool.tile([128, S], F16, tag="fk")
                nc.vector.tensor_mul(out=fq, in0=yst[:, 0, :], in1=bcq)
                nc.vector.tensor_mul(out=fk, in0=yst[:, 1, :], in1=bck)
            else:
                yst = fpool.tile([128, S], F16, tag="yst")
                _evict(l, yst[:, :], mm2[:, :], aall[:, b2c:b2c + 1])
                nc.vector.tensor_mul(out=y2st[0:Co, :], in0=yst[0:Co, :],
                                     in1=yst[0:Co, :])
                nc.vector.tensor_mul(out=y2st[64:64 + Co, :],
                                     in0=yst[64:64 + Co, :],
                                     in1=yst[64:64 + Co, :])
                ssq = psm.tile([2, S], F32, tag="sm")
                sc = SEL2[l]
                nc.tensor.matmul(ssq, lhsT=hall[:, sc:sc + 2], rhs=y2st,
                                 start=True, stop=True)
                t1 = work.tile([2, S], F32, tag="t1")
                nc.scalar.activation(out=t1, in_=ssq, func=AF.Ln)
                rn = work.tile([2, S], F16, tag="rn")
                nc.scalar.activation(out=rn, in_=t1, func=AF.Exp,
                                     scale=-0.5, bias=aall[0:2, RNC:RNC + 1])
                bc = psm.tile([128, S], F32, tag="sm")
                bs = BSEL[l]
                nc.tensor.matmul(bc, lhsT=hrow[0:2, bs:bs + 128], rhs=rn,
                                 start=True, stop=True)
                fq = fpool.tile([128, S], F16, tag="fq")
                fk = fpool.tile([128, S], F16, tag="fk")
                nc.vector.tensor_mul(out=fq[0:Co, :], in0=yst[0:Co, :],
                                     in1=bc[0:Co, :])
                nc.vector.tensor_mul(out=fk[0:Co, :], in0=yst[64:64 + Co, :],
                                     in1=bc[64:64 + Co, :])
            pprod = work.tile([128, S], F16, tag="pp")
            nc.vector.tensor_mul(out=pprod[0:Co, :], in0=fq[0:Co, :],
                                 in1=fk[0:Co, :])
            jp = work.tile([128, S], F16, tag="jp")
            nc.vector.tensor_scalar(
                out=jp[0:Co, :], in0=pprod[0:Co, :], scalar1=1.0, scalar2=0.0,
                op0=ALU.mult, op1=ALU.add,
                accum_out=catp[0:Co, l:l + 1])
            st[(l, 'f')] = (fq, fk)

        def emit_tail2(l):
            Co = COUT[l]
            fq, fk = st[(l, 'f')]
            g = pg.tile([128, 4, S], F32, tag="g")
            for m in range(4):
                nc.tensor.matmul(
                    g[:, m, :],
                    lhsT=fk[0:Co, m * 128:(m + 1) * 128],
                    rhs=fq[0:Co, :],
                    start=True, stop=True)
            E = epool.tile([128, 4, S], F16, tag="E")
            nc.scalar.activation(out=E, in_=g, func=AF.Exp,
                                 scale=1.0, bias=aall[:, BEC:BEC + 1])
            Z = psm.tile([1, S], F32, tag="sm")
            for m in range(4):
                nc.tensor.matmul(Z, lhsT=hall[:, 0:1], rhs=E[:, m, :],
                                 start=(m == 0), stop=(m == 3))
            jl = work.tile([1, S], F32, tag="jl")
            nc.scalar.activation(out=jl, in_=Z, func=AF.Ln,
                                 accum_out=catb[:, l:l + 1])

        # schedule: software-pipelined across layers (LORD = 1,3,2,0)
        emit_subs(1)
        emit_mlp1(1)
        emit_mlp2(1)
        emit_subs(3)
        emit_tail1(1)
        emit_mlp1(3)
        emit_tail2(1)
        emit_mlp2(3)
        emit_tail1(3)
        emit_subs(2)
        emit_mlp1(2)
        emit_tail2(3)
        emit_mlp2(2)
        emit_subs(0)
        emit_tail1(2)
        emit_mlp1(0)
        emit_tail2(2)
        emit_mlp2(0)
        emit_tail1(0)
        emit_tail2(0)

        # finale: res = sum_l sum_s lnZ' - sum l_pos' + COMB_C
        pmmf = psm.tile([1, 4], F32, tag="sm")
        nc.tensor.matmul(pmmf, lhsT=aall[:, OCOL:OCOL + 1], rhs=catp,
                         start=True, stop=True)
        tpos = const.tile([1, 1], F32)
        nc.vector.reduce_sum(out=tpos, in_=pmmf, axis=mybir.AxisListType.X)
        tlnz = const.tile([1, 1], F32)
        nc.vector.reduce_sum(out=tlnz, in_=catb[:, 0:4],
                             axis=mybir.AxisListType.X)
        res = const.tile([1, 1], F32)
        nc.vector.tensor_scalar(out=res, in0=tlnz, scalar1=tpos,
                                scalar2=COMB_C, op0=ALU.subtract, op1=ALU.add)
        nc.sync.dma_start(out=out[:, :], in_=res[:, :])
    nc.finalize()
    return nc


_NC_CACHE = {}


def _get_nc(key="v2", **kw):
    if key not in _NC_CACHE:
        _NC_CACHE[key] = _build_nc(**kw)
    return _NC_CACHE[key]


def _host_blobs(inputs):
    nidx, cidx = [], []
    for l in range(4):
        sid = np.asarray(inputs[f"sid{l}"]).astype(np.int64)
        nidx.append(((sid[:, 0:1] + _DH) * 32 + (sid[:, 1:2] + _DW)).reshape(-1))
        cidx.append(np.repeat((sid[:, 0] + 1) * 32 + (sid[:, 1] + 1), 8))

    wts = np.zeros((128, WTOT), dtype=np.float16)
    aux = np.zeros((128, AUXW), dtype=np.float32)
    for l in range(4):
        w1T = np.asarray(inputs[f"w1_{l}"]).astype(np.float32).T
        w2T = np.asarray(inputs[f"w2_{l}"]).astype(np.float32).T
        b1 = np.asarray(inputs[f"b1_{l}"]).astype(np.float32)
        b2 = np.asarray(inputs[f"b2_{l}"]).astype(np.float32)
        C, Co = CS[l], COUT[l]
        for k in range(KC[l]):
            rows = min(128, C - k * 128)
            c0 = W1C[(l, k)]
            wts[0:rows, c0:c0 + C] = w1T[k * 128:k * 128 + rows, :]
            c0 = W2C[(l, k)]
            wts[0:rows, c0:c0 + Co] = w2T[k * 128:k * 128 + rows, :]
        for m in range(KC[l]):
            rows = min(128, C - m * 128)
            aux[0:rows, B1C[(l, m)]] = b1[m * 128:m * 128 + rows]
        if l == 3:
            aux[0:Co, B2C[l]] = b2
        else:
            aux[0:Co, B2C[l]] = b2
            aux[64:64 + Co, B2C[l]] = b2
    aux[:, OCOL] = 1.0
    aux[:, RNC] = BIAS_RN
    aux[:, BEC] = BIAS_E

    auxh = np.zeros((128, 8), dtype=np.float16)
    auxh[:, 0] = 1.0
    for l in range(3):
        Co = COUT[l]
        auxh[0:Co, SEL2[l]] = 1.0
        auxh[64:64 + Co, SEL2[l] + 1] = 1.0
    auxh2 = np.zeros((2, 512), dtype=np.float16)
    auxh2[0, 0:128] = 1.0
    for l in range(3):
        Co = COUT[l]
        auxh2[0, BSEL[l]:BSEL[l] + Co] = 1.0
        auxh2[1, BSEL[l] + 64:BSEL[l] + 64 + Co] = 1.0

    xqs = [np.zeros((NCH * 128, 1024), dtype=np.float16) for _ in range(NCORES)]
    xks = [np.zeros((NCH * 128, 1024), dtype=np.float16) for _ in range(NCORES)]
    for l in range(4):
        C = CS[l]
        fq = np.asarray(inputs[f"fq{l}"])[:, :, :32, :32].reshape(NCORES, C, 1024)
        fk = np.asarray(inputs[f"fk{l}"])[:, :, :32, :32].reshape(NCORES, C, 1024)
        qn = fq[:, :, nidx[l]]
        qc = fq[:, :, cidx[l]]
        kn = fk[:, :, nidx[l]]
        kc_ = fk[:, :, cidx[l]]
        for b in range(NCORES):
            for k in range(KC[l]):
                r0 = CH[(l, k)] * 128
                rows = min(128, C - k * 128)
                sl = slice(k * 128, k * 128 + rows)
                xqs[b][r0:r0 + rows, 0:512] = qn[b, sl, :]
                xqs[b][r0:r0 + rows, 512:1024] = qc[b, sl, :]
                xks[b][r0:r0 + rows, 0:512] = kn[b, sl, :]
                xks[b][r0:r0 + rows, 512:1024] = kc_[b, sl, :]
    return wts, aux, auxh, auxh2, xqs, xks


_LAST_RESULT = {}


def kernel(**inputs):
    assert int(inputs.get("start_layer", 0)) == 0
    assert int(inputs.get("end_layer", 4)) == 4
    assert int(inputs.get("num_s", 64)) == 64

    nc = _get_nc()
    wts, aux, auxh, auxh2, xqs, xks = _host_blobs(inputs)
    in_maps = [
        {"xq": xqs[b], "xk": xks[b], "wts": wts, "aux": aux,
         "auxh": auxh, "auxh2": auxh2}
        for b in range(NCORES)
    ]
    r = run_bass_kernel_spmd(nc, in_maps, core_ids=list(range(NCORES)))
    _LAST_RESULT["r"] = r
    partials = [np.float64(r.results[b]["out"][0, 0]) for b in range(NCORES)]
    loss = np.float32(sum(partials) / (NCORES * S))
    return np.asarray(loss, dtype=np.float32)


# revision 11
# speedup vs baseline: 1.3402x; 1.3402x over previous
"""CCPL contrastive loss kernel for Trainium2 (8 NeuronCores, SPMD data-parallel).

Contract: kernel(**inputs) takes FULL unsharded inputs, returns the FULL scalar
loss (float32, shape ()).  Host does indexing-only gathers (sid in [0,30) means
only the top-left 32x32 corner of every feature map is read); core b processes
batch b end-to-end on device; host sums the 8 partial CE sums / (8*512).

v4 design (vs 64.2us baseline):
 - x uploaded partition-major [128, NCH*1024] with the center duplicated 8x:
   few large DMA descriptors, and x = neigh - center is a flat fp16 SBUF sub
   (DVE 2x mode); x split across the gpsimd + scalar DGE queues
 - f' = y * rn with rn = exp(-0.5 ln ssq - 0.5 ln tau): both branches carry
   1/sqrt(tau) so the Gram G' = fq'^T fk' is already G/tau
 - Gram computed transposed (G'[t-block, s]) as 4 single-bank PSUM tiles per
   layer from a 2-slot ring; exp per tile (ACT, bias -0.5/tau keeps E' fp16);
   Z row-sums as ones^T E' PE matmuls accumulating over the 4 t-blocks
 - sum_s ln Z via ACT Ln with accum_out; sum_s l_pos via fp16 pprod + DVE
   tensor_scalar accum_out (no big DVE reduces anywhere)
 - l0-2: q/k band-stacked in partitions [0:Co]/[64:64+Co] for mm2/ssq/bc
   (selector matmuls); l3 mm2 as two 1-bank tiles
 - PSUM = mm1 ring (2x1) + small ring (4x1) + gram ring (2x1) = 8 banks
 - tails of the last three layers emitted micro-phase round-robin so their
   serial dependency chains overlap each other and the remaining MLP work
"""

import numpy as np
from contextlib import ExitStack

import concourse.bass as bass
import concourse.bacc as bacc
import concourse.tile as tile
from concourse import mybir
from concourse.bass_utils import run_bass_kernel_spmd

F32 = mybir.dt.float32
F16 = mybir.dt.float16
ALU = mybir.AluOpType
AF = mybir.ActivationFunctionType

# Force Exp/Ln/Relu into the one table set containing all three so the kernel
# pays a single ACT_TABLE_LOAD.
_COMBINED_SET = "natural_log_exp_and_others"
_orig_get_tables = bacc.get_activation_tables


def _patched_get_tables(arch):
    t = _orig_get_tables(arch)
    strip = {AF.Exp, AF.Ln, AF.Relu}
    return {
        name: (fns if name == _COMBINED_SET else (set(fns) - strip))
        for name, fns in t.items()
    }


bacc.get_activation_tables = _patched_get_tables

TAU = 0.07
NCORES = 8
S = 512
CS = [64, 128, 256, 512]
COUT = [16, 32, 64, 128]
KC = [1, 1, 2, 4]
NCH = sum(KC)
LORD = (1, 3, 2, 0)          # processing order (also x/wts blob order)
TRIO = (3, 2, 0)             # tail-interleaved layers
_DH = np.array([0, 0, 0, 1, 1, 2, 2, 2], dtype=np.int64)
_DW = np.array([0, 1, 2, 0, 2, 0, 1, 2], dtype=np.int64)
BIAS_RN = -0.5 * float(np.log(TAU))   # rn' = exp(-0.5 ln ssq + BIAS_RN)
BIAS_E = -0.5 / TAU                   # exp(G' - 0.5/tau)
COMB_C = 4 * S * 0.5 / TAU            # restores the BIAS_E shift in sum_s lnZ

# chunk indices in LORD order ------------------------------------------------
CH = {}
_c = 0
for _l in LORD:
    for _k in range(KC[_l]):
        CH[(_l, _k)] = _c
        _c += 1

# weight blob column offsets, LORD order so DMA slices are contiguous
W1C, W2C, WBLK = {}, {}, {}
_c = 0
for _l in LORD:
    _s = _c
    for _k in range(KC[_l]):
        W1C[(_l, _k)] = _c
        _c += CS[_l]
    for _k in range(KC[_l]):
        W2C[(_l, _k)] = _c
        _c += COUT[_l]
    WBLK[_l] = (_s, _c)
WTOT = _c

# aux blob (f32): cols 0..7 b1 chunks, 8..11 b2 (band-stacked for l<3), 12 ones
B1C = {}
_c = 0
for _l in range(4):
    for _m in range(KC[_l]):
        B1C[(_l, _m)] = _c
        _c += 1
B2C = {l: 8 + l for l in range(4)}
OCOL = 12
RNC = 13          # BIAS_RN column
BEC = 14          # BIAS_E column
AUXW = 16
# auxh (f16) [128, 8]: col 0 ones column; cols 1+2l / 2+2l: ssq band selectors
# auxh2 (f16) [2, 512]: cols 0:128 ones row (row 0); bsel_l at 128+128l
SEL2 = {l: 1 + 2 * l for l in range(3)}
BSEL = {l: 128 + 128 * l for l in range(3)}

# engine assignment knobs ----------------------------------------------------
RELU_ENG = {}
for _l in range(4):
    for _m in range(KC[_l]):
        for _b in range(2):
            RELU_ENG[(_l, _m, _b)] = 'a' if (_m + _b) % 2 == 0 else 'v'
EVICT_ENG = {0: 'v', 1: 'v', 2: 'v', 3: 'v'}


def _build_nc(relu_eng=None, evict_eng=None):
    relu_eng = relu_eng or RELU_ENG
    evict_eng = evict_eng or EVICT_ENG
    nc = bacc.Bacc()
    xq = nc.dram_tensor("xq", [128, NCH * 1024], F16, kind="ExternalInput")
    xk = nc.dram_tensor("xk", [128, NCH * 1024], F16, kind="ExternalInput")
    wts = nc.dram_tensor("wts", [128, WTOT], F16, kind="ExternalInput")
    aux = nc.dram_tensor("aux", [128, AUXW], F32, kind="ExternalInput")
    auxh = nc.dram_tensor("auxh", [128, 8], F16, kind="ExternalInput")
    auxh2 = nc.dram_tensor("auxh2", [2, 512], F16, kind="ExternalInput")
    out = nc.dram_tensor("out", [1, 1], F32, kind="ExternalOutput")

    with ExitStack() as ctx:
        tc = ctx.enter_context(tile.TileContext(nc))
        const = ctx.enter_context(tc.tile_pool(name="const", bufs=1))
        work = ctx.enter_context(tc.tile_pool(name="work", bufs=3))
        fpool = ctx.enter_context(tc.tile_pool(name="fpool", bufs=3))
        epool = ctx.enter_context(tc.tile_pool(name="epool", bufs=3))
        # PSUM: mm1 (2x1 bank) + sm (4x1 bank) + g (2x1 bank) = 8 banks
        pmm = ctx.enter_context(tc.tile_pool(name="pmm", bufs=2, space="PSUM"))
        psm = ctx.enter_context(tc.tile_pool(name="psm", bufs=4, space="PSUM"))
        pg = ctx.enter_context(tc.tile_pool(name="pg", bufs=2, space="PSUM"))

        xq_s = const.tile([128, NCH, 1024], F16)
        xk_s = const.tile([128, NCH, 1024], F16)
        wall = const.tile([128, WTOT], F16)
        aall = const.tile([128, AUXW], F32)
        hall = const.tile([128, 8], F16)
        hrow = const.tile([2, 512], F16)
        xall = (xq_s, xk_s)
        xdr = (xq, xk)

        def xdma(eng, b, c0, c1):
            eng.dma_start(out=xall[b][:, c0:c1, :],
                          in_=xdr[b][:, c0 * 1024:c1 * 1024])

        # l1 + l3q + first half of l3k on gpsimd; rest of x on scalar
        c1, _ = CH[(1, 0)], None
        c3 = CH[(3, 0)]
        c2 = CH[(2, 0)]
        c0_ = CH[(0, 0)]
        xdma(nc.gpsimd, 0, c1, c1 + 1)
        xdma(nc.gpsimd, 1, c1, c1 + 1)
        xdma(nc.gpsimd, 0, c3, c3 + 2)
        xdma(nc.gpsimd, 0, c3 + 2, c3 + 4)
        xdma(nc.gpsimd, 1, c3, c3 + 2)
        xdma(nc.scalar, 1, c3 + 2, c3 + 4)
        xdma(nc.scalar, 0, c2, c2 + 2)
        xdma(nc.scalar, 1, c2, c2 + 2)
        xdma(nc.scalar, 0, c0_, c0_ + 1)
        xdma(nc.scalar, 1, c0_, c0_ + 1)
        w0, w1_ = WBLK[LORD[0]]
        nc.sync.dma_start(out=wall[:, w0:w1_], in_=wts[:, w0:w1_])
        nc.sync.dma_start(out=hall, in_=auxh[:, :])
        nc.sync.dma_start(out=hrow, in_=auxh2[:, :])
        nc.sync.dma_start(out=aall, in_=aux[:, :])
        for l in LORD[1:]:
            w0, w1_ = WBLK[l]
            nc.sync.dma_start(out=wall[:, w0:w1_], in_=wts[:, w0:w1_])

        # shared state
        xsub = [const.tile([128, NCH, S], F16, tag=f"xsub{b}",
                           name=f"xsub{b}") for b in range(2)]
        # per-layer band-stacked y^2 tiles (pre-zeroed; sharing one tile
        # across layers corrupts a layer's ssq when tails interleave)
        y2st = {l: const.tile([128, S], F16, tag=f"y2st{l}", name=f"y2st{l}")
                for l in (0, 1, 2)}
        catb = const.tile([1, 8], F32)         # per-layer sum_s lnZ'
        catp = const.tile([128, 4], F32)       # per-layer banded sum_s l_pos'
        for l in (0, 1, 2):
            nc.gpsimd.memset(y2st[l][:, :], 0.0)
        nc.gpsimd.memset(catb[:, :], 0.0)
        nc.gpsimd.memset(catp[:, :], 0.0)

        st = {}   # per-layer tiles carried between phases

        def emit_subs(l, branches=(0, 1)):
            a0, a1 = CH[(l, 0)], CH[(l, 0)] + KC[l]
            step = 2 if l == 3 else (a1 - a0)
            for b in branches:
                for c in range(a0, a1, step):
                    nc.vector.tensor_sub(
                        out=xsub[b][:, c:c + step, :],
                        in0=xall[b][:, c:c + step, 0:512],
                        in1=xall[b][:, c:c + step, 512:1024])

        def emit_mlp1(l, branches=(0, 1)):
            C, Co, K = CS[l], COUT[l], KC[l]
            if (l, 'h') not in st:
                st[(l, 'h')] = const.tile([128, K, 2, S], F16, tag=f"h{l}",
                                          name=f"h{l}")
            h = st[(l, 'h')]
            for b in branches:
                for m in range(K):
                    rows = min(128, C - m * 128)
                    mm1 = pmm.tile([128, S], F32, tag="mm1")
                    for k in range(K):
                        c0 = W1C[(l, k)] + m * 128
                        nc.tensor.matmul(
                            mm1[0:rows, :],
                            lhsT=wall[:, c0:c0 + rows],
                            rhs=xsub[b][:, CH[(l, k)], :],
                            start=(k == 0), stop=(k == K - 1))
                    bc1 = B1C[(l, m)]
                    if relu_eng[(l, m, b)] == 'a':
                        nc.scalar.activation(
                            out=h[0:rows, m, b, :], in_=mm1[0:rows, :],
                            func=AF.Relu, bias=aall[0:rows, bc1:bc1 + 1],
                            scale=1.0)
                    else:
                        nc.vector.tensor_scalar(
                            out=h[0:rows, m, b, :], in0=mm1[0:rows, :],
                            scalar1=aall[0:rows, bc1:bc1 + 1], scalar2=0.0,
                            op0=ALU.add, op1=ALU.max)

        def emit_mlp2(l):
            C, Co, K = CS[l], COUT[l], KC[l]
            h = st[(l, 'h')]
            if l == 3:
                mm2 = [psm.tile([128, S], F32, tag="sm", name=f"mm2q{l}"),
                       psm.tile([128, S], F32, tag="sm", name=f"mm2k{l}")]
                for b in range(2):
                    for k in range(K):
                        c0 = W2C[(l, k)]
                        nc.tensor.matmul(
                            mm2[b][:, :],
                            lhsT=wall[0:128, c0:c0 + Co],
                            rhs=h[:, k, b, :],
                            start=(k == 0), stop=(k == K - 1))
            else:
                mm2 = psm.tile([128, S], F32, tag="sm", name=f"mm2st{l}")
                for b in range(2):
                    for k in range(K):
                        rows = min(128, C - k * 128)
                        c0 = W2C[(l, k)]
                        nc.tensor.matmul(
                            mm2[64 * b:64 * b + Co, :],
                            lhsT=wall[0:rows, c0:c0 + Co],
                            rhs=h[0:rows, k, b, :],
                            start=(k == 0), stop=(k == K - 1))
            st[(l, 'mm2')] = mm2

        def _ts_add(eng, outt, int_, bias_ap):
            if eng == 'a':
                nc.scalar.activation(out=outt, in_=int_, func=AF.Relu,
                                     bias=bias_ap, scale=1.0)  # unused
            else:
                nc.vector.tensor_scalar_add(out=outt, in0=int_,
                                            scalar1=bias_ap)

        # ---- tail micro-phases (t_*: evict .. pos, g_*: gram/exp/Z/lnZ) ----
        def t_evict(l):
            Co = COUT[l]
            mm2 = st[(l, 'mm2')]
            b2c = B2C[l]
            if l == 3:
                yst = fpool.tile([128, 2, S], F16, tag="yst3")
                for b in range(2):
                    nc.vector.tensor_scalar_add(
                        out=yst[:, b, :], in0=mm2[b][:, :],
                        scalar1=aall[:, b2c:b2c + 1])
            else:
                yst = fpool.tile([128, S], F16, tag="yst")
                nc.vector.tensor_scalar_add(out=yst[:, :], in0=mm2[:, :],
                                            scalar1=aall[:, b2c:b2c + 1])
            st[(l, 'yst')] = yst

        def t_y2(l):
            Co = COUT[l]
            yst = st[(l, 'yst')]
            if l == 3:
                y2q = work.tile([128, S], F16, tag="y2q")
                y2k = work.tile([128, S], F16, tag="y2k")
                nc.vector.tensor_mul(out=y2q, in0=yst[:, 0, :], in1=yst[:, 0, :])
                nc.vector.tensor_mul(out=y2k, in0=yst[:, 1, :], in1=yst[:, 1, :])
                st[(l, 'y2')] = (y2q, y2k)
            else:
                nc.vector.tensor_mul(out=y2st[l][0:Co, :], in0=yst[0:Co, :],
                                     in1=yst[0:Co, :])
                nc.vector.tensor_mul(out=y2st[l][64:64 + Co, :],
                                     in0=yst[64:64 + Co, :],
                                     in1=yst[64:64 + Co, :])

        def t_ssq(l):
            if l == 3:
                y2q, y2k = st[(l, 'y2')]
                ssq_q = psm.tile([1, S], F32, tag="sm", name="ssq3q")
                ssq_k = psm.tile([1, S], F32, tag="sm", name="ssq3k")
                nc.tensor.matmul(ssq_q, lhsT=hall[:, 0:1], rhs=y2q,
                                 start=True, stop=True)
                nc.tensor.matmul(ssq_k, lhsT=hall[:, 0:1], rhs=y2k,
                                 start=True, stop=True)
                st[(l, 'ssq')] = (ssq_q, ssq_k)
            else:
                ssq = psm.tile([2, S], F32, tag="sm", name=f"ssq{l}")
                sc = SEL2[l]
                nc.tensor.matmul(ssq, lhsT=hall[:, sc:sc + 2], rhs=y2st[l],
                                 start=True, stop=True)
                st[(l, 'ssq')] = ssq

        def t_ln(l):
            if l == 3:
                ssq_q, ssq_k = st[(l, 'ssq')]
                t3q = work.tile([1, S], F32, tag="t3q")
                t3k = work.tile([1, S], F32, tag="t3k")
                nc.scalar.activation(out=t3q, in_=ssq_q, func=AF.Ln)
                nc.scalar.activation(out=t3k, in_=ssq_k, func=AF.Ln)
                st[(l, 't1')] = (t3q, t3k)
            else:
                t1 = work.tile([2, S], F32, tag="t1")
                nc.scalar.activation(out=t1, in_=st[(l, 'ssq')], func=AF.Ln)
                st[(l, 't1')] = t1

        def t_rn(l):
            if l == 3:
                t3q, t3k = st[(l, 't1')]
                rnq = work.tile([1, S], F16, tag="rnq")
                rnk = work.tile([1, S], F16, tag="rnk")
                nc.scalar.activation(out=rnq, in_=t3q, func=AF.Exp,
                                     scale=-0.5, bias=aall[0:1, RNC:RNC + 1])
                nc.scalar.activation(out=rnk, in_=t3k, func=AF.Exp,
                                     scale=-0.5, bias=aall[0:1, RNC:RNC + 1])
                st[(l, 'rn')] = (rnq, rnk)
            else:
                rn = work.tile([2, S], F16, tag="rn")
                nc.scalar.activation(out=rn, in_=st[(l, 't1')], func=AF.Exp,
                                     scale=-0.5, bias=aall[0:2, RNC:RNC + 1])
                st[(l, 'rn')] = rn

        def t_bc(l):
            if l == 3:
                rnq, rnk = st[(l, 'rn')]
                bcq = psm.tile([128, S], F32, tag="sm", name="bc3q")
                bck = psm.tile([128, S], F32, tag="sm", name="bc3k")
                nc.tensor.matmul(bcq, lhsT=hrow[0:1, 0:128], rhs=rnq,
                                 start=True, stop=True)
                nc.tensor.matmul(bck, lhsT=hrow[0:1, 0:128], rhs=rnk,
                                 start=True, stop=True)
                st[(l, 'bc')] = (bcq, bck)
            else:
                bc = psm.tile([128, S], F32, tag="sm", name=f"bc{l}")
                bs = BSEL[l]
                nc.tensor.matmul(bc, lhsT=hrow[0:2, bs:bs + 128],
                                 rhs=st[(l, 'rn')], start=True, stop=True)
                st[(l, 'bc')] = bc

        def t_f(l):
            Co = COUT[l]
            yst = st[(l, 'yst')]
            fq = fpool.tile([128, S], F16, tag="fq")
            fk = fpool.tile([128, S], F16, tag="fk")
            if l == 3:
                bcq, bck = st[(l, 'bc')]
                nc.vector.tensor_mul(out=fq, in0=yst[:, 0, :], in1=bcq)
                nc.vector.tensor_mul(out=fk, in0=yst[:, 1, :], in1=bck)
            else:
                bc = st[(l, 'bc')]
                nc.vector.tensor_mul(out=fq[0:Co, :], in0=yst[0:Co, :],
                                     in1=bc[0:Co, :])
                nc.vector.tensor_mul(out=fk[0:Co, :], in0=yst[64:64 + Co, :],
                                     in1=bc[64:64 + Co, :])
            st[(l, 'f')] = (fq, fk)

        def t_pos(l):
            Co = COUT[l]
            fq, fk = st[(l, 'f')]
            pprod = work.tile([128, S], F16, tag="pp")
            nc.vector.tensor_mul(out=pprod[0:Co, :], in0=fq[0:Co, :],
                                 in1=fk[0:Co, :])
            jp = work.tile([128, S], F16, tag="jp")
            nc.vector.tensor_scalar(
                out=jp[0:Co, :], in0=pprod[0:Co, :], scalar1=1.0, scalar2=0.0,
                op0=ALU.mult, op1=ALU.add,
                accum_out=catp[0:Co, l:l + 1])

        def g_gram(l, m):
            Co = COUT[l]
            fq, fk = st[(l, 'f')]
            g = pg.tile([128, S], F32, tag="g", name=f"g{l}_{m}")
            nc.tensor.matmul(g, lhsT=fk[0:Co, m * 128:(m + 1) * 128],
                             rhs=fq[0:Co, :], start=True, stop=True)
            st[(l, 'g', m)] = g

        def g_exp(l, m):
            if (l, 'E') not in st:
                st[(l, 'E')] = epool.tile([128, 4, S], F16, tag="E",
                                          name=f"E{l}")
            E = st[(l, 'E')]
            nc.scalar.activation(out=E[:, m, :], in_=st.pop((l, 'g', m)),
                                 func=AF.Exp, scale=1.0,
                                 bias=aall[:, BEC:BEC + 1])

        def g_z(l, m):
            if (l, 'Z') not in st:
                st[(l, 'Z')] = psm.tile([1, S], F32, tag="sm", name=f"Z{l}")
            E = st[(l, 'E')]
            nc.tensor.matmul(st[(l, 'Z')], lhsT=hall[:, 0:1], rhs=E[:, m, :],
                             start=(m == 0), stop=(m == 3))

        def g_lnz(l):
            jl = work.tile([1, S], F32, tag="jl")
            nc.scalar.activation(out=jl, in_=st[(l, 'Z')], func=AF.Ln,
                                 accum_out=catb[:, l:l + 1])

        def emit_tail(l):
            for ph in (t_evict, t_y2, t_ssq, t_ln, t_rn, t_bc, t_f, t_pos):
                ph(l)
            for m in range(4):
                g_gram(l, m)
                g_exp(l, m)
                g_z(l, m)
            g_lnz(l)

        # ---------------- schedule ----------------
        emit_subs(1)
        emit_mlp1(1)
        emit_mlp2(1)
        emit_subs(3, branches=(0,))
        t_evict(1); t_y2(1); t_ssq(1); t_ln(1); t_rn(1); t_bc(1); t_f(1)
        emit_mlp1(3, branches=(0,))
        t_pos(1)
        for m in range(4):
            g_gram(1, m)
            g_exp(1, m)
            g_z(1, m)
        g_lnz(1)
        emit_subs(3, branches=(1,))
        emit_mlp1(3, branches=(1,))
        emit_subs(2)
        emit_mlp1(2)
        emit_mlp2(3)
        emit_mlp2(2)
        emit_subs(0)
        emit_mlp1(0)
        emit_mlp2(0)
        # interleave the three remaining tails micro-phase round-robin
        for ph in (t_evict, t_y2, t_ssq, t_ln, t_rn, t_bc, t_f):
            for l in TRIO:
                ph(l)
        for m in range(4):
            for l in TRIO:
                g_gram(l, m)
                g_exp(l, m)
                g_z(l, m)
        for l in TRIO:
            t_pos(l)
            g_lnz(l)

        # finale: res = sum_l sum_s lnZ' - sum l_pos' + COMB_C
        pmmf = psm.tile([1, 4], F32, tag="sm", name="pmmf")
        nc.tensor.matmul(pmmf, lhsT=aall[:, OCOL:OCOL + 1], rhs=catp,
                         start=True, stop=True)
        tpos = const.tile([1, 1], F32)
        nc.vector.reduce_sum(out=tpos, in_=pmmf, axis=mybir.AxisListType.X)
        tlnz = const.tile([1, 1], F32)
        nc.vector.reduce_sum(out=tlnz, in_=catb[:, 0:4],
                             axis=mybir.AxisListType.X)
        res = const.tile([1, 1], F32)
        nc.vector.tensor_scalar(out=res, in0=tlnz, scalar1=tpos,
                                scalar2=COMB_C, op0=ALU.subtract, op1=ALU.add)
        nc.sync.dma_start(out=out[:, :], in_=res[:, :])
    nc.finalize()
    return nc


_NC_CACHE = {}


def _get_nc(key="v4", **kw):
    if key not in _NC_CACHE:
        _NC_CACHE[key] = _build_nc(**kw)
    return _NC_CACHE[key]


def _host_blobs(inputs):
    nidx, cidx = [], []
    for l in range(4):
        sid = np.asarray(inputs[f"sid{l}"]).astype(np.int64)
        nidx.append(((sid[:, 0:1] + _DH) * 32 + (sid[:, 1:2] + _DW)).reshape(-1))
        cidx.append(np.repeat((sid[:, 0] + 1) * 32 + (sid[:, 1] + 1), 8))

    wts = np.zeros((128, WTOT), dtype=np.float16)
    aux = np.zeros((128, AUXW), dtype=np.float32)
    for l in range(4):
        w1T = np.asarray(inputs[f"w1_{l}"]).astype(np.float32).T
        w2T = np.asarray(inputs[f"w2_{l}"]).astype(np.float32).T
        b1 = np.asarray(inputs[f"b1_{l}"]).astype(np.float32)
        b2 = np.asarray(inputs[f"b2_{l}"]).astype(np.float32)
        C, Co = CS[l], COUT[l]
        for k in range(KC[l]):
            rows = min(128, C - k * 128)
            c0 = W1C[(l, k)]
            wts[0:rows, c0:c0 + C] = w1T[k * 128:k * 128 + rows, :]
            c0 = W2C[(l, k)]
            wts[0:rows, c0:c0 + Co] = w2T[k * 128:k * 128 + rows, :]
        for m in range(KC[l]):
            rows = min(128, C - m * 128)
            aux[0:rows, B1C[(l, m)]] = b1[m * 128:m * 128 + rows]
        aux[0:Co, B2C[l]] = b2
        if l != 3:
            aux[64:64 + Co, B2C[l]] = b2
    aux[:, OCOL] = 1.0
    aux[:, RNC] = BIAS_RN
    aux[:, BEC] = BIAS_E

    auxh = np.zeros((128, 8), dtype=np.float16)
    auxh[:, 0] = 1.0
    for l in range(3):
        Co = COUT[l]
        auxh[0:Co, SEL2[l]] = 1.0
        auxh[64:64 + Co, SEL2[l] + 1] = 1.0
    auxh2 = np.zeros((2, 512), dtype=np.float16)
    auxh2[0, 0:128] = 1.0
    for l in range(3):
        Co = COUT[l]
        auxh2[0, BSEL[l]:BSEL[l] + Co] = 1.0
        auxh2[1, BSEL[l] + 64:BSEL[l] + 64 + Co] = 1.0

    # partition-major x blobs: [128, NCH*1024], chunk n at cols n*1024,
    # [neigh 512 | center-dup 512] per chunk
    xqs = [np.zeros((128, NCH * 1024), dtype=np.float16) for _ in range(NCORES)]
    xks = [np.zeros((128, NCH * 1024), dtype=np.float16) for _ in range(NCORES)]
    for l in range(4):
        C = CS[l]
        fq = np.asarray(inputs[f"fq{l}"])[:, :, :32, :32].reshape(NCORES, C, 1024)
        fk = np.asarray(inputs[f"fk{l}"])[:, :, :32, :32].reshape(NCORES, C, 1024)
        qn = fq[:, :, nidx[l]]
        qc = fq[:, :, cidx[l]]
        kn = fk[:, :, nidx[l]]
        kc_ = fk[:, :, cidx[l]]
        for b in range(NCORES):
            for k in range(KC[l]):
                n0 = CH[(l, k)] * 1024
                rows = min(128, C - k * 128)
                sl = slice(k * 128, k * 128 + rows)
                xqs[b][0:rows, n0:n0 + 512] = qn[b, sl, :]
                xqs[b][0:rows, n0 + 512:n0 + 1024] = qc[b, sl, :]
                xks[b][0:rows, n0:n0 + 512] = kn[b, sl, :]
                xks[b][0:rows, n0 + 512:n0 + 1024] = kc_[b, sl, :]
    return wts, aux, auxh, auxh2, xqs, xks


_LAST_RESULT = {}


def kernel(**inputs):
    assert int(inputs.get("start_layer", 0)) == 0
    assert int(inputs.get("end_layer", 4)) == 4
    assert int(inputs.get("num_s", 64)) == 64

    nc = _get_nc()
    wts, aux, auxh, auxh2, xqs, xks = _host_blobs(inputs)
    in_maps = [
        {"xq": xqs[b], "xk": xks[b], "wts": wts, "aux": aux,
         "auxh": auxh, "auxh2": auxh2}
        for b in range(NCORES)
    ]
    r = run_bass_kernel_spmd(nc, in_maps, core_ids=list(range(NCORES)))
    _LAST_RESULT["r"] = r
    partials = [np.float64(r.results[b]["out"][0, 0]) for b in range(NCORES)]
    loss = np.float32(sum(partials) / (NCORES * S))
    return np.asarray(loss, dtype=np.float32)


# revision 12
# speedup vs baseline: 1.3555x; 1.0114x over previous
"""CCPL contrastive loss kernel for Trainium2 (8 NeuronCores, SPMD data-parallel).

Contract: kernel(**inputs) takes FULL unsharded inputs, returns the FULL scalar
loss (float32, shape ()).  Host does indexing-only gathers (sid in [0,30) means
only the top-left 32x32 corner of every feature map is read); core b processes
batch b end-to-end on device; host sums the 8 partial CE sums / (8*512).

v4 design (vs 64.2us baseline):
 - x uploaded partition-major [128, NCH*1024] with the center duplicated 8x:
   few large DMA descriptors, and x = neigh - center is a flat fp16 SBUF sub
   (DVE 2x mode); x split across the gpsimd + scalar DGE queues
 - f' = y * rn with rn = exp(-0.5 ln ssq - 0.5 ln tau): both branches carry
   1/sqrt(tau) so the Gram G' = fq'^T fk' is already G/tau
 - Gram computed transposed (G'[t-block, s]) as 4 single-bank PSUM tiles per
   layer from a 2-slot ring; exp per tile (ACT, bias -0.5/tau keeps E' fp16);
   Z row-sums as ones^T E' PE matmuls accumulating over the 4 t-blocks
 - sum_s ln Z via ACT Ln with accum_out; sum_s l_pos via fp16 pprod + DVE
   tensor_scalar accum_out (no big DVE reduces anywhere)
 - l0-2: q/k band-stacked in partitions [0:Co]/[64:64+Co] for mm2/ssq/bc
   (selector matmuls); l3 mm2 as two 1-bank tiles
 - PSUM = mm1 ring (2x1) + small ring (4x1) + gram ring (2x1) = 8 banks
 - tails of the last three layers emitted micro-phase round-robin so their
   serial dependency chains overlap each other and the remaining MLP work
"""

import numpy as np
from contextlib import ExitStack

import concourse.bass as bass
import concourse.bacc as bacc
import concourse.tile as tile
from concourse import mybir
from concourse.bass_utils import run_bass_kernel_spmd

F32 = mybir.dt.float32
F16 = mybir.dt.float16
ALU = mybir.AluOpType
AF = mybir.ActivationFunctionType

# Force Exp/Ln/Relu into the one table set containing all three so the kernel
# pays a single ACT_TABLE_LOAD.
_COMBINED_SET = "natural_log_exp_and_others"
_orig_get_tables = bacc.get_activation_tables


def _patched_get_tables(arch):
    t = _orig_get_tables(arch)
    return {
        name: (fns if name == _COMBINED_SET else set())
        for name, fns in t.items()
    }


bacc.get_activation_tables = _patched_get_tables

TAU = 0.07
NCORES = 8
S = 512
CS = [64, 128, 256, 512]
COUT = [16, 32, 64, 128]
KC = [1, 1, 2, 4]
NCH = sum(KC)
LORD = (1, 3, 2, 0)          # processing order (also x/wts blob order)
TRIO = (3, 2, 0)             # tail-interleaved layers
_DH = np.array([0, 0, 0, 1, 1, 2, 2, 2], dtype=np.int64)
_DW = np.array([0, 1, 2, 0, 2, 0, 1, 2], dtype=np.int64)
BIAS_RN = -0.5 * float(np.log(TAU))   # rn' = exp(-0.5 ln ssq + BIAS_RN)
BIAS_E = -0.5 / TAU                   # exp(G' - 0.5/tau)
COMB_C = 4 * S * 0.5 / TAU            # restores the BIAS_E shift in sum_s lnZ

# chunk indices in LORD order ------------------------------------------------
CH = {}
_c = 0
for _l in LORD:
    for _k in range(KC[_l]):
        CH[(_l, _k)] = _c
        _c += 1

# weight blob column offsets, LORD order so DMA slices are contiguous
W1C, W2C, WBLK = {}, {}, {}
_c = 0
for _l in LORD:
    _s = _c
    for _k in range(KC[_l]):
        W1C[(_l, _k)] = _c
        _c += CS[_l]
    for _k in range(KC[_l]):
        W2C[(_l, _k)] = _c
        _c += COUT[_l]
    WBLK[_l] = (_s, _c)
WTOT = _c

# aux blob (f32): cols 0..7 b1 chunks, 8..11 b2 (band-stacked for l<3), 12 ones
B1C = {}
_c = 0
for _l in range(4):
    for _m in range(KC[_l]):
        B1C[(_l, _m)] = _c
        _c += 1
B2C = {l: 8 + l for l in range(4)}
OCOL = 12
RNC = 13          # BIAS_RN column
BEC = 14          # BIAS_E column
WVC = 16          # [1]*16 | [-1]*4 final-combine weights (row 0)
AUXW = 36
# auxh (f16) [128, 8]: col 0 ones column; cols 1+2l / 2+2l: ssq band selectors
# auxh2 (f16) [2, 512]: cols 0:128 ones row (row 0); bsel_l at 128+128l
SEL2 = {l: 1 + 2 * l for l in range(3)}
BSEL = {l: 128 + 128 * l for l in range(3)}

# engine assignment knobs ----------------------------------------------------
RELU_ENG = {}
for _l in range(4):
    for _m in range(KC[_l]):
        for _b in range(2):
            RELU_ENG[(_l, _m, _b)] = 'a' if (_m + _b) % 2 == 0 else 'v'
EVICT_ENG = {0: 'v', 1: 'v', 2: 'v', 3: 'v'}


def _build_nc(relu_eng=None, evict_eng=None):
    relu_eng = relu_eng or RELU_ENG
    evict_eng = evict_eng or EVICT_ENG
    nc = bacc.Bacc()
    xq = nc.dram_tensor("xq", [128, NCH * 1024], F16, kind="ExternalInput")
    xk = nc.dram_tensor("xk", [128, NCH * 1024], F16, kind="ExternalInput")
    wts = nc.dram_tensor("wts", [128, WTOT], F16, kind="ExternalInput")
    aux = nc.dram_tensor("aux", [128, AUXW], F32, kind="ExternalInput")
    auxh = nc.dram_tensor("auxh", [128, 8], F16, kind="ExternalInput")
    auxh2 = nc.dram_tensor("auxh2", [2, 512], F16, kind="ExternalInput")
    out = nc.dram_tensor("out", [1, 1], F32, kind="ExternalOutput")

    with ExitStack() as ctx:
        tc = ctx.enter_context(tile.TileContext(nc))
        const = ctx.enter_context(tc.tile_pool(name="const", bufs=1))
        work = ctx.enter_context(tc.tile_pool(name="work", bufs=3))
        fpool = ctx.enter_context(tc.tile_pool(name="fpool", bufs=3))
        # PSUM: mm1 (2x1 bank) + sm (4x1 bank) + g (2x1 bank) = 8 banks
        pmm = ctx.enter_context(tc.tile_pool(name="pmm", bufs=2, space="PSUM"))
        psm = ctx.enter_context(tc.tile_pool(name="psm", bufs=4, space="PSUM"))
        pg = ctx.enter_context(tc.tile_pool(name="pg", bufs=2, space="PSUM"))

        xq_s = const.tile([128, NCH, 1024], F16)
        xk_s = const.tile([128, NCH, 1024], F16)
        wall = const.tile([128, WTOT], F16)
        aall = const.tile([128, AUXW], F32)
        hall = const.tile([128, 8], F16)
        hrow = const.tile([2, 512], F16)
        xall = (xq_s, xk_s)
        xdr = (xq, xk)

        def xdma(eng, b, c0, c1):
            eng.dma_start(out=xall[b][:, c0:c1, :],
                          in_=xdr[b][:, c0 * 1024:c1 * 1024])

        # l1 + l3q + first half of l3k on gpsimd; rest of x on scalar
        c1, _ = CH[(1, 0)], None
        c3 = CH[(3, 0)]
        c2 = CH[(2, 0)]
        c0_ = CH[(0, 0)]
        xdma(nc.gpsimd, 0, c1, c1 + 1)
        xdma(nc.gpsimd, 1, c1, c1 + 1)
        xdma(nc.gpsimd, 0, c3, c3 + 2)
        xdma(nc.gpsimd, 1, c3, c3 + 2)
        xdma(nc.gpsimd, 0, c0_, c0_ + 1)
        xdma(nc.gpsimd, 1, c0_, c0_ + 1)
        xdma(nc.scalar, 0, c3 + 2, c3 + 4)
        xdma(nc.scalar, 1, c3 + 2, c3 + 4)
        xdma(nc.scalar, 0, c2, c2 + 2)
        xdma(nc.scalar, 1, c2, c2 + 2)
        w0, w1_ = WBLK[LORD[0]]
        nc.sync.dma_start(out=wall[:, w0:w1_], in_=wts[:, w0:w1_])
        nc.sync.dma_start(out=hall, in_=auxh[:, :])
        nc.sync.dma_start(out=hrow, in_=auxh2[:, :])
        nc.sync.dma_start(out=aall, in_=aux[:, :])
        for l in LORD[1:]:
            w0, w1_ = WBLK[l]
            nc.sync.dma_start(out=wall[:, w0:w1_], in_=wts[:, w0:w1_])

        # shared state
        xsub = [const.tile([128, NCH, S], F16, tag=f"xsub{b}",
                           name=f"xsub{b}") for b in range(2)]
        # per-layer band-stacked y^2 tiles (pre-zeroed; sharing one tile
        # across layers corrupts a layer's ssq when tails interleave)
        y2st = {l: const.tile([128, S], F16, tag=f"y2st{l}", name=f"y2st{l}")
                for l in (0, 1, 2)}
        ZD = const.tile([128, 16], F32)        # Z per (s-block, layer*4+m)
        # cols 0..15: ln(ZD); cols 16..19: per-layer banded sum_s l_pos'
        catL = const.tile([128, 20], F32)
        for l in (0, 1, 2):
            nc.gpsimd.memset(y2st[l][:, :], 0.0)
        nc.gpsimd.memset(catL[:, :], 0.0)

        st = {}   # per-layer tiles carried between phases

        def emit_subs(l, branches=(0, 1)):
            a0, a1 = CH[(l, 0)], CH[(l, 0)] + KC[l]
            step = 2 if l == 3 else (a1 - a0)
            for b in branches:
                for c in range(a0, a1, step):
                    nc.vector.tensor_sub(
                        out=xsub[b][:, c:c + step, :],
                        in0=xall[b][:, c:c + step, 0:512],
                        in1=xall[b][:, c:c + step, 512:1024])

        def emit_mlp1(l, branches=(0, 1)):
            C, Co, K = CS[l], COUT[l], KC[l]
            if (l, 'h') not in st:
                st[(l, 'h')] = const.tile([128, K, 2, S], F16, tag=f"h{l}",
                                          name=f"h{l}")
            h = st[(l, 'h')]
            for b in branches:
                for m in range(K):
                    rows = min(128, C - m * 128)
                    mm1 = pmm.tile([128, S], F32, tag="mm1")
                    for k in range(K):
                        c0 = W1C[(l, k)] + m * 128
                        nc.tensor.matmul(
                            mm1[0:rows, :],
                            lhsT=wall[:, c0:c0 + rows],
                            rhs=xsub[b][:, CH[(l, k)], :],
                            start=(k == 0), stop=(k == K - 1))
                    bc1 = B1C[(l, m)]
                    if relu_eng[(l, m, b)] == 'a':
                        nc.scalar.activation(
                            out=h[0:rows, m, b, :], in_=mm1[0:rows, :],
                            func=AF.Relu, bias=aall[0:rows, bc1:bc1 + 1],
                            scale=1.0)
                    else:
                        nc.vector.tensor_scalar(
                            out=h[0:rows, m, b, :], in0=mm1[0:rows, :],
                            scalar1=aall[0:rows, bc1:bc1 + 1], scalar2=0.0,
                            op0=ALU.add, op1=ALU.max)

        def emit_mlp2(l):
            C, Co, K = CS[l], COUT[l], KC[l]
            h = st[(l, 'h')]
            if l == 3:
                mm2 = [psm.tile([128, S], F32, tag="sm", name=f"mm2q{l}"),
                       psm.tile([128, S], F32, tag="sm", name=f"mm2k{l}")]
                for b in range(2):
                    for k in range(K):
                        c0 = W2C[(l, k)]
                        nc.tensor.matmul(
                            mm2[b][:, :],
                            lhsT=wall[0:128, c0:c0 + Co],
                            rhs=h[:, k, b, :],
                            start=(k == 0), stop=(k == K - 1))
            else:
                mm2 = psm.tile([128, S], F32, tag="sm", name=f"mm2st{l}")
                for b in range(2):
                    for k in range(K):
                        rows = min(128, C - k * 128)
                        c0 = W2C[(l, k)]
                        nc.tensor.matmul(
                            mm2[64 * b:64 * b + Co, :],
                            lhsT=wall[0:rows, c0:c0 + Co],
                            rhs=h[0:rows, k, b, :],
                            start=(k == 0), stop=(k == K - 1))
            st[(l, 'mm2')] = mm2

        def _ts_add(eng, outt, int_, bias_ap):
            if eng == 'a':
                nc.scalar.activation(out=outt, in_=int_, func=AF.Relu,
                                     bias=bias_ap, scale=1.0)  # unused
            else:
                nc.vector.tensor_scalar_add(out=outt, in0=int_,
                                            scalar1=bias_ap)

        # ---- tail micro-phases (t_*: evict .. pos, g_*: gram/exp/Z/lnZ) ----
        def t_evict(l):
            Co = COUT[l]
            mm2 = st[(l, 'mm2')]
            b2c = B2C[l]
            if l == 3:
                yst = fpool.tile([128, 2, S], F16, tag="yst3")
                for b in range(2):
                    nc.vector.tensor_scalar_add(
                        out=yst[:, b, :], in0=mm2[b][:, :],
                        scalar1=aall[:, b2c:b2c + 1])
            else:
                yst = fpool.tile([128, S], F16, tag="yst")
                nc.vector.tensor_scalar_add(out=yst[:, :], in0=mm2[:, :],
                                            scalar1=aall[:, b2c:b2c + 1])
            st[(l, 'yst')] = yst

        def t_y2(l):
            Co = COUT[l]
            yst = st[(l, 'yst')]
            if l == 3:
                y2q = work.tile([128, S], F16, tag="y2q")
                y2k = work.tile([128, S], F16, tag="y2k")
                nc.vector.tensor_mul(out=y2q, in0=yst[:, 0, :], in1=yst[:, 0, :])
                nc.vector.tensor_mul(out=y2k, in0=yst[:, 1, :], in1=yst[:, 1, :])
                st[(l, 'y2')] = (y2q, y2k)
            else:
                nc.vector.tensor_mul(out=y2st[l][0:Co, :], in0=yst[0:Co, :],
                                     in1=yst[0:Co, :])
                nc.vector.tensor_mul(out=y2st[l][64:64 + Co, :],
                                     in0=yst[64:64 + Co, :],
                                     in1=yst[64:64 + Co, :])

        def t_ssq(l):
            if l == 3:
                y2q, y2k = st[(l, 'y2')]
                ssq_q = psm.tile([1, S], F32, tag="sm", name="ssq3q")
                ssq_k = psm.tile([1, S], F32, tag="sm", name="ssq3k")
                nc.tensor.matmul(ssq_q, lhsT=hall[:, 0:1], rhs=y2q,
                                 start=True, stop=True)
                nc.tensor.matmul(ssq_k, lhsT=hall[:, 0:1], rhs=y2k,
                                 start=True, stop=True)
                st[(l, 'ssq')] = (ssq_q, ssq_k)
            else:
                ssq = psm.tile([2, S], F32, tag="sm", name=f"ssq{l}")
                sc = SEL2[l]
                nc.tensor.matmul(ssq, lhsT=hall[:, sc:sc + 2], rhs=y2st[l],
                                 start=True, stop=True)
                st[(l, 'ssq')] = ssq

        def t_ln(l):
            if l == 3:
                ssq_q, ssq_k = st[(l, 'ssq')]
                t3q = work.tile([1, S], F32, tag="t3q")
                t3k = work.tile([1, S], F32, tag="t3k")
                nc.scalar.activation(out=t3q, in_=ssq_q, func=AF.Ln)
                nc.scalar.activation(out=t3k, in_=ssq_k, func=AF.Ln)
                st[(l, 't1')] = (t3q, t3k)
            else:
                t1 = work.tile([2, S], F32, tag="t1")
                nc.scalar.activation(out=t1, in_=st[(l, 'ssq')], func=AF.Ln)
                st[(l, 't1')] = t1

        def t_rn(l):
            if l == 3:
                t3q, t3k = st[(l, 't1')]
                rnq = work.tile([1, S], F16, tag="rnq")
                rnk = work.tile([1, S], F16, tag="rnk")
                nc.scalar.activation(out=rnq, in_=t3q, func=AF.Exp,
                                     scale=-0.5, bias=aall[0:1, RNC:RNC + 1])
                nc.scalar.activation(out=rnk, in_=t3k, func=AF.Exp,
                                     scale=-0.5, bias=aall[0:1, RNC:RNC + 1])
                st[(l, 'rn')] = (rnq, rnk)
            else:
                rn = work.tile([2, S], F16, tag="rn")
                nc.scalar.activation(out=rn, in_=st[(l, 't1')], func=AF.Exp,
                                     scale=-0.5, bias=aall[0:2, RNC:RNC + 1])
                st[(l, 'rn')] = rn

        def t_bc(l):
            if l == 3:
                rnq, rnk = st[(l, 'rn')]
                bcq = psm.tile([128, S], F32, tag="sm", name="bc3q")
                bck = psm.tile([128, S], F32, tag="sm", name="bc3k")
                nc.tensor.matmul(bcq, lhsT=hrow[0:1, 0:128], rhs=rnq,
                                 start=True, stop=True)
                nc.tensor.matmul(bck, lhsT=hrow[0:1, 0:128], rhs=rnk,
                                 start=True, stop=True)
                st[(l, 'bc')] = (bcq, bck)
            else:
                bc = psm.tile([128, S], F32, tag="sm", name=f"bc{l}")
                bs = BSEL[l]
                nc.tensor.matmul(bc, lhsT=hrow[0:2, bs:bs + 128],
                                 rhs=st[(l, 'rn')], start=True, stop=True)
                st[(l, 'bc')] = bc

        def t_f(l):
            Co = COUT[l]
            yst = st[(l, 'yst')]
            fq = fpool.tile([128, S], F16, tag="fq")
            fk = fpool.tile([128, S], F16, tag="fk")
            if l == 3:
                bcq, bck = st[(l, 'bc')]
                nc.vector.tensor_mul(out=fq, in0=yst[:, 0, :], in1=bcq)
                nc.vector.tensor_mul(out=fk, in0=yst[:, 1, :], in1=bck)
            else:
                bc = st[(l, 'bc')]
                nc.vector.tensor_mul(out=fq[0:Co, :], in0=yst[0:Co, :],
                                     in1=bc[0:Co, :])
                nc.vector.tensor_mul(out=fk[0:Co, :], in0=yst[64:64 + Co, :],
                                     in1=bc[64:64 + Co, :])
            st[(l, 'f')] = (fq, fk)

        def t_pos(l):
            Co = COUT[l]
            fq, fk = st[(l, 'f')]
            pprod = work.tile([128, S], F16, tag="pp")
            nc.vector.tensor_mul(out=pprod[0:Co, :], in0=fq[0:Co, :],
                                 in1=fk[0:Co, :])
            jp = work.tile([128, S], F16, tag="jp")
            nc.vector.tensor_scalar(
                out=jp[0:Co, :], in0=pprod[0:Co, :], scalar1=1.0, scalar2=0.0,
                op0=ALU.mult, op1=ALU.add,
                accum_out=catL[0:Co, 16 + l:17 + l])

        def g_gram(l, m):
            Co = COUT[l]
            fq, fk = st[(l, 'f')]
            g = pg.tile([128, S], F32, tag="g", name=f"g{l}_{m}")
            nc.tensor.matmul(g, lhsT=fq[0:Co, m * 128:(m + 1) * 128],
                             rhs=fk[0:Co, :], start=True, stop=True)
            st[(l, 'g', m)] = g

        def g_exp(l, m):
            # exp in place on the PSUM tile; the row sum (over t) goes
            # straight to ZD via the ACT accumulator
            g = st.pop((l, 'g', m))
            i = 4 * l + m
            nc.scalar.activation(out=g, in_=g, func=AF.Exp, scale=1.0,
                                 bias=aall[:, BEC:BEC + 1],
                                 accum_out=ZD[:, i:i + 1])

        # ---------------- schedule ----------------
        emit_subs(1)
        emit_mlp1(1)
        emit_mlp2(1)
        emit_subs(3, branches=(0,))
        t_evict(1); t_y2(1); t_ssq(1); t_ln(1); t_rn(1); t_bc(1); t_f(1)
        emit_mlp1(3, branches=(0,))
        t_pos(1)
        for m in range(4):
            g_gram(1, m)
            g_exp(1, m)
        emit_subs(3, branches=(1,))
        emit_mlp1(3, branches=(1,))
        emit_subs(2)
        emit_mlp1(2)
        emit_mlp2(3)
        emit_mlp2(2)
        emit_subs(0)
        emit_mlp1(0)
        emit_mlp2(0)
        # interleave the three remaining tails micro-phase round-robin
        for ph in (t_evict, t_y2, t_ssq, t_ln, t_rn, t_bc, t_f):
            for l in TRIO:
                ph(l)
        for m in range(4):
            for l in TRIO:
                g_gram(l, m)
                g_exp(l, m)
        for l in TRIO:
            t_pos(l)

        # finale: res = sum ln ZD - sum l_pos' + COMB_C
        nc.scalar.activation(out=catL[:, 0:16], in_=ZD[:, :], func=AF.Ln)
        pmmf = psm.tile([1, 20], F32, tag="sm", name="pmmf")
        nc.tensor.matmul(pmmf, lhsT=aall[:, OCOL:OCOL + 1], rhs=catL,
                         start=True, stop=True)
        wprod = const.tile([1, 20], F32)
        nc.vector.tensor_mul(out=wprod, in0=pmmf,
                             in1=aall[0:1, WVC:WVC + 20])
        r0 = const.tile([1, 1], F32)
        nc.vector.reduce_sum(out=r0, in_=wprod, axis=mybir.AxisListType.X)
        res = const.tile([1, 1], F32)
        nc.vector.tensor_scalar(out=res, in0=r0, scalar1=1.0,
                                scalar2=COMB_C, op0=ALU.mult, op1=ALU.add)
        nc.sync.dma_start(out=out[:, :], in_=res[:, :])
    nc.finalize()
    return nc


_NC_CACHE = {}


def _get_nc(key="v4", **kw):
    if key not in _NC_CACHE:
        _NC_CACHE[key] = _build_nc(**kw)
    return _NC_CACHE[key]


def _host_blobs(inputs):
    nidx, cidx = [], []
    for l in range(4):
        sid = np.asarray(inputs[f"sid{l}"]).astype(np.int64)
        nidx.append(((sid[:, 0:1] + _DH) * 32 + (sid[:, 1:2] + _DW)).reshape(-1))
        cidx.append(np.repeat((sid[:, 0] + 1) * 32 + (sid[:, 1] + 1), 8))

    wts = np.zeros((128, WTOT), dtype=np.float16)
    aux = np.zeros((128, AUXW), dtype=np.float32)
    for l in range(4):
        w1T = np.asarray(inputs[f"w1_{l}"]).astype(np.float32).T
        w2T = np.asarray(inputs[f"w2_{l}"]).astype(np.float32).T
        b1 = np.asarray(inputs[f"b1_{l}"]).astype(np.float32)
        b2 = np.asarray(inputs[f"b2_{l}"]).astype(np.float32)
        C, Co = CS[l], COUT[l]
        for k in range(KC[l]):
            rows = min(128, C - k * 128)
            c0 = W1C[(l, k)]
            wts[0:rows, c0:c0 + C] = w1T[k * 128:k * 128 + rows, :]
            c0 = W2C[(l, k)]
            wts[0:rows, c0:c0 + Co] = w2T[k * 128:k * 128 + rows, :]
        for m in range(KC[l]):
            rows = min(128, C - m * 128)
            aux[0:rows, B1C[(l, m)]] = b1[m * 128:m * 128 + rows]
        aux[0:Co, B2C[l]] = b2
        if l != 3:
            aux[64:64 + Co, B2C[l]] = b2
    aux[:, OCOL] = 1.0
    aux[:, RNC] = BIAS_RN
    aux[:, BEC] = BIAS_E
    aux[0, WVC:WVC + 16] = 1.0
    aux[0, WVC + 16:WVC + 20] = -1.0

    auxh = np.zeros((128, 8), dtype=np.float16)
    auxh[:, 0] = 1.0
    for l in range(3):
        Co = COUT[l]
        auxh[0:Co, SEL2[l]] = 1.0
        auxh[64:64 + Co, SEL2[l] + 1] = 1.0
    auxh2 = np.zeros((2, 512), dtype=np.float16)
    auxh2[0, 0:128] = 1.0
    for l in range(3):
        Co = COUT[l]
        auxh2[0, BSEL[l]:BSEL[l] + Co] = 1.0
        auxh2[1, BSEL[l] + 64:BSEL[l] + 64 + Co] = 1.0

    # partition-major x blobs: [128, NCH*1024], chunk n at cols n*1024,
    # [neigh 512 | center-dup 512] per chunk
    xqs = [np.zeros((128, NCH * 1024), dtype=np.float16) for _ in range(NCORES)]
    xks = [np.zeros((128, NCH * 1024), dtype=np.float16) for _ in range(NCORES)]
    for l in range(4):
        C = CS[l]
        fq = np.asarray(inputs[f"fq{l}"])[:, :, :32, :32].reshape(NCORES, C, 1024)
        fk = np.asarray(inputs[f"fk{l}"])[:, :, :32, :32].reshape(NCORES, C, 1024)
        qn = fq[:, :, nidx[l]]
        qc = fq[:, :, cidx[l]]
        kn = fk[:, :, nidx[l]]
        kc_ = fk[:, :, cidx[l]]
        for b in range(NCORES):
            for k in range(KC[l]):
                n0 = CH[(l, k)] * 1024
                rows = min(128, C - k * 128)
                sl = slice(k * 128, k * 128 + rows)
                xqs[b][0:rows, n0:n0 + 512] = qn[b, sl, :]
                xqs[b][0:rows, n0 + 512:n0 + 1024] = qc[b, sl, :]
                xks[b][0:rows, n0:n0 + 512] = kn[b, sl, :]
                xks[b][0:rows, n0 + 512:n0 + 1024] = kc_[b, sl, :]
    return wts, aux, auxh, auxh2, xqs, xks


_LAST_RESULT = {}


def kernel(**inputs):
    assert int(inputs.get("start_layer", 0)) == 0
    assert int(inputs.get("end_layer", 4)) == 4
    assert int(inputs.get("num_s", 64)) == 64

    nc = _get_nc()
    wts, aux, auxh, auxh2, xqs, xks = _host_blobs(inputs)
    in_maps = [
        {"xq": xqs[b], "xk": xks[b], "wts": wts, "aux": aux,
         "auxh": auxh, "auxh2": auxh2}
        for b in range(NCORES)
    ]
    r = run_bass_kernel_spmd(nc, in_maps, core_ids=list(range(NCORES)))
    _LAST_RESULT["r"] = r
    partials = [np.float64(r.results[b]["out"][0, 0]) for b in range(NCORES)]
    loss = np.float32(sum(partials) / (NCORES * S))
    return np.asarray(loss, dtype=np.float32)


# revision 13
# speedup vs baseline: 1.5325x; 1.1306x over previous
"""CCPL contrastive loss kernel for Trainium2 (8 NeuronCores, SPMD data-parallel).

Contract: kernel(**inputs) takes FULL unsharded inputs, returns the FULL scalar
loss (float32, shape ()).  Host does indexing-only gathers (sid in [0,30) means
only the top-left 32x32 corner of every feature map is read); core b processes
batch b end-to-end on device; host sums the 8 partial CE sums / (8*512).

v4 design (vs 64.2us baseline):
 - x uploaded partition-major [128, NCH*1024] with the center duplicated 8x:
   few large DMA descriptors, and x = neigh - center is a flat fp16 SBUF sub
   (DVE 2x mode); x split across the gpsimd + scalar DGE queues
 - f' = y * rn with rn = exp(-0.5 ln ssq - 0.5 ln tau): both branches carry
   1/sqrt(tau) so the Gram G' = fq'^T fk' is already G/tau
 - Gram computed transposed (G'[t-block, s]) as 4 single-bank PSUM tiles per
   layer from a 2-slot ring; exp per tile (ACT, bias -0.5/tau keeps E' fp16);
   Z row-sums as ones^T E' PE matmuls accumulating over the 4 t-blocks
 - sum_s ln Z via ACT Ln with accum_out; sum_s l_pos via fp16 pprod + DVE
   tensor_scalar accum_out (no big DVE reduces anywhere)
 - l0-2: q/k band-stacked in partitions [0:Co]/[64:64+Co] for mm2/ssq/bc
   (selector matmuls); l3 mm2 as two 1-bank tiles
 - PSUM = mm1 ring (2x1) + small ring (4x1) + gram ring (2x1) = 8 banks
 - tails of the last three layers emitted micro-phase round-robin so their
   serial dependency chains overlap each other and the remaining MLP work
"""

import numpy as np
from contextlib import ExitStack

import concourse.bass as bass
import concourse.bacc as bacc
import concourse.tile as tile
from concourse import mybir
from concourse.bass_utils import run_bass_kernel_spmd

F32 = mybir.dt.float32
F16 = mybir.dt.float16
F8 = mybir.dt.float8e4
ALU = mybir.AluOpType
AF = mybir.ActivationFunctionType

# Force Exp/Ln/Relu into the one table set containing all three so the kernel
# pays a single ACT_TABLE_LOAD.
_COMBINED_SET = "natural_log_exp_and_others"
_orig_get_tables = bacc.get_activation_tables


def _patched_get_tables(arch):
    t = _orig_get_tables(arch)
    return {
        name: (fns if name == _COMBINED_SET else set())
        for name, fns in t.items()
    }


bacc.get_activation_tables = _patched_get_tables

TAU = 0.07
NCORES = 8
S = 512
CS = [64, 128, 256, 512]
COUT = [16, 32, 64, 128]
KC = [1, 1, 2, 4]
NCH = sum(KC)
LORD = (1, 3, 2, 0)          # processing order (also x/wts blob order)
TRIO = (3, 2, 0)             # tail-interleaved layers
_DH = np.array([0, 0, 0, 1, 1, 2, 2, 2], dtype=np.int64)
_DW = np.array([0, 1, 2, 0, 2, 0, 1, 2], dtype=np.int64)
BIAS_RN = -0.5 * float(np.log(TAU))   # rn' = exp(-0.5 ln ssq + BIAS_RN)
BIAS_E = -0.5 / TAU                   # exp(G' - 0.5/tau)
COMB_C = 4 * S * 0.5 / TAU            # restores the BIAS_E shift in sum_s lnZ

# chunk indices in LORD order ------------------------------------------------
CH = {}
_c = 0
for _l in LORD:
    for _k in range(KC[_l]):
        CH[(_l, _k)] = _c
        _c += 1

# weight blob column offsets, LORD order so DMA slices are contiguous
W1C, W2C, WBLK = {}, {}, {}
_c = 0
for _l in LORD:
    _s = _c
    for _k in range(KC[_l]):
        W1C[(_l, _k)] = _c
        _c += CS[_l]
    for _k in range(KC[_l]):
        W2C[(_l, _k)] = _c
        _c += COUT[_l]
    WBLK[_l] = (_s, _c)
WTOT = _c

# aux blob (f32): cols 0..7 b1 chunks, 8..11 b2 (band-stacked for l<3), 12 ones
B1C = {}
_c = 0
for _l in range(4):
    for _m in range(KC[_l]):
        B1C[(_l, _m)] = _c
        _c += 1
B2C = {l: 8 + l for l in range(4)}
OCOL = 12
RNC = 13          # BIAS_RN column
BEC = 14          # BIAS_E column
WVC = 16          # [1]*16 | [-1]*4 final-combine weights (row 0)
AUXW = 36
# auxh (f16) [128, 8]: col 0 ones column; cols 1+2l / 2+2l: ssq band selectors
# auxh2 (f16) [2, 512]: cols 0:128 ones row (row 0); bsel_l at 128+128l
SEL2 = {l: 1 + 2 * l for l in range(3)}
BSEL = {l: 128 + 128 * l for l in range(3)}

# engine assignment knobs ----------------------------------------------------
RELU_ENG = {}
for _l in range(4):
    for _m in range(KC[_l]):
        for _b in range(2):
            RELU_ENG[(_l, _m, _b)] = 'a' if (_m + _b) % 2 == 0 else 'v'
EVICT_ENG = {0: 'v', 1: 'v', 2: 'v', 3: 'v'}


def _build_nc(relu_eng=None, evict_eng=None):
    relu_eng = relu_eng or RELU_ENG
    evict_eng = evict_eng or EVICT_ENG
    nc = bacc.Bacc()
    xq = nc.dram_tensor("xq", [128, NCH * 576], F8, kind="ExternalInput")
    xk = nc.dram_tensor("xk", [128, NCH * 576], F8, kind="ExternalInput")
    wts = nc.dram_tensor("wts", [128, WTOT], F16, kind="ExternalInput")
    aux = nc.dram_tensor("aux", [128, AUXW], F32, kind="ExternalInput")
    auxh = nc.dram_tensor("auxh", [128, 8], F16, kind="ExternalInput")
    auxh2 = nc.dram_tensor("auxh2", [2, 512], F16, kind="ExternalInput")
    out = nc.dram_tensor("out", [1, 1], F32, kind="ExternalOutput")

    with ExitStack() as ctx:
        tc = ctx.enter_context(tile.TileContext(nc))
        const = ctx.enter_context(tc.tile_pool(name="const", bufs=1))
        work = ctx.enter_context(tc.tile_pool(name="work", bufs=3))
        fpool = ctx.enter_context(tc.tile_pool(name="fpool", bufs=3))
        # PSUM: mm1 (2x1 bank) + sm (4x1 bank) + g (2x1 bank) = 8 banks
        pmm = ctx.enter_context(tc.tile_pool(name="pmm", bufs=2, space="PSUM"))
        psm = ctx.enter_context(tc.tile_pool(name="psm", bufs=4, space="PSUM"))
        pg = ctx.enter_context(tc.tile_pool(name="pg", bufs=2, space="PSUM"))

        xq_s = const.tile([128, NCH, 576], F8)
        xk_s = const.tile([128, NCH, 576], F8)
        wall = const.tile([128, WTOT], F16)
        aall = const.tile([128, AUXW], F32)
        hall = const.tile([128, 8], F16)
        hrow = const.tile([2, 512], F16)
        xall = (xq_s, xk_s)
        xdr = (xq, xk)

        def xdma(eng, b, c0, c1):
            eng.dma_start(out=xall[b][:, c0:c1, :],
                          in_=xdr[b][:, c0 * 576:c1 * 576])

        # l1 + l3q + first half of l3k on gpsimd; rest of x on scalar
        c1, _ = CH[(1, 0)], None
        c3 = CH[(3, 0)]
        c2 = CH[(2, 0)]
        c0_ = CH[(0, 0)]
        xdma(nc.gpsimd, 0, c1, c1 + 1)
        xdma(nc.gpsimd, 1, c1, c1 + 1)
        xdma(nc.gpsimd, 0, c3, c3 + 2)
        xdma(nc.gpsimd, 1, c3, c3 + 2)
        xdma(nc.gpsimd, 0, c0_, c0_ + 1)
        xdma(nc.gpsimd, 1, c0_, c0_ + 1)
        xdma(nc.scalar, 0, c3 + 2, c3 + 4)
        xdma(nc.scalar, 1, c3 + 2, c3 + 4)
        xdma(nc.scalar, 0, c2, c2 + 2)
        xdma(nc.scalar, 1, c2, c2 + 2)
        w0, w1_ = WBLK[LORD[0]]
        nc.sync.dma_start(out=wall[:, w0:w1_], in_=wts[:, w0:w1_])
        nc.sync.dma_start(out=hall, in_=auxh[:, :])
        nc.sync.dma_start(out=hrow, in_=auxh2[:, :])
        nc.sync.dma_start(out=aall, in_=aux[:, :])
        for l in LORD[1:]:
            w0, w1_ = WBLK[l]
            nc.sync.dma_start(out=wall[:, w0:w1_], in_=wts[:, w0:w1_])

        # shared state
        xsub = [const.tile([128, NCH, S], F16, tag=f"xsub{b}",
                           name=f"xsub{b}") for b in range(2)]
        # per-layer band-stacked y^2 tiles (pre-zeroed; sharing one tile
        # across layers corrupts a layer's ssq when tails interleave)
        y2st = {l: const.tile([128, S], F16, tag=f"y2st{l}", name=f"y2st{l}")
                for l in (0, 1, 2)}
        ZD = const.tile([128, 16], F32)        # Z per (s-block, layer*4+m)
        # cols 0..15: ln(ZD); cols 16..19: per-layer banded sum_s l_pos'
        catL = const.tile([128, 20], F32)
        for l in (0, 1, 2):
            nc.gpsimd.memset(y2st[l][:, :], 0.0)
        nc.gpsimd.memset(catL[:, :], 0.0)

        st = {}   # per-layer tiles carried between phases

        def emit_subs(l, branches=(0, 1)):
            a0, a1 = CH[(l, 0)], CH[(l, 0)] + KC[l]
            for b in branches:
                in0 = xall[b][:, a0:a1, 0:512].rearrange(
                    "p n (s j) -> p n s j", j=8)
                cb = xall[b][:, a0:a1, 512:576]
                in1 = bass.AP(cb.tensor, cb.offset, [*cb.ap, [0, 8]])
                nc.vector.tensor_sub(
                    out=xsub[b][:, a0:a1, :].rearrange(
                        "p n (s j) -> p n s j", j=8),
                    in0=in0, in1=in1)

        def emit_mlp1(l, branches=(0, 1)):
            C, Co, K = CS[l], COUT[l], KC[l]
            if (l, 'h') not in st:
                st[(l, 'h')] = const.tile([128, K, 2, S], F16, tag=f"h{l}",
                                          name=f"h{l}")
            h = st[(l, 'h')]
            for b in branches:
                for m in range(K):
                    rows = min(128, C - m * 128)
                    mm1 = pmm.tile([128, S], F32, tag="mm1")
                    for k in range(K):
                        c0 = W1C[(l, k)] + m * 128
                        nc.tensor.matmul(
                            mm1[0:rows, :],
                            lhsT=wall[:, c0:c0 + rows],
                            rhs=xsub[b][:, CH[(l, k)], :],
                            start=(k == 0), stop=(k == K - 1))
                    bc1 = B1C[(l, m)]
                    if relu_eng[(l, m, b)] == 'a':
                        nc.scalar.activation(
                            out=h[0:rows, m, b, :], in_=mm1[0:rows, :],
                            func=AF.Relu, bias=aall[0:rows, bc1:bc1 + 1],
                            scale=1.0)
                    else:
                        nc.vector.tensor_scalar(
                            out=h[0:rows, m, b, :], in0=mm1[0:rows, :],
                            scalar1=aall[0:rows, bc1:bc1 + 1], scalar2=0.0,
                            op0=ALU.add, op1=ALU.max)

        def emit_mlp2(l):
            C, Co, K = CS[l], COUT[l], KC[l]
            h = st[(l, 'h')]
            if l == 3:
                mm2 = [psm.tile([128, S], F32, tag="sm", name=f"mm2q{l}"),
                       psm.tile([128, S], F32, tag="sm", name=f"mm2k{l}")]
                for b in range(2):
                    for k in range(K):
                        c0 = W2C[(l, k)]
                        nc.tensor.matmul(
                            mm2[b][:, :],
                            lhsT=wall[0:128, c0:c0 + Co],
                            rhs=h[:, k, b, :],
                            start=(k == 0), stop=(k == K - 1))
            else:
                mm2 = psm.tile([128, S], F32, tag="sm", name=f"mm2st{l}")
                for b in range(2):
                    for k in range(K):
                        rows = min(128, C - k * 128)
                        c0 = W2C[(l, k)]
                        nc.tensor.matmul(
                            mm2[64 * b:64 * b + Co, :],
                            lhsT=wall[0:rows, c0:c0 + Co],
                            rhs=h[0:rows, k, b, :],
                            start=(k == 0), stop=(k == K - 1))
            st[(l, 'mm2')] = mm2

        def _ts_add(eng, outt, int_, bias_ap):
            if eng == 'a':
                nc.scalar.activation(out=outt, in_=int_, func=AF.Relu,
                                     bias=bias_ap, scale=1.0)  # unused
            else:
                nc.vector.tensor_scalar_add(out=outt, in0=int_,
                                            scalar1=bias_ap)

        # ---- tail micro-phases (t_*: evict .. pos, g_*: gram/exp/Z/lnZ) ----
        def t_evict(l):
            Co = COUT[l]
            mm2 = st[(l, 'mm2')]
            b2c = B2C[l]
            if l == 3:
                yst = fpool.tile([128, 2, S], F16, tag="yst3")
                for b in range(2):
                    nc.vector.tensor_scalar_add(
                        out=yst[:, b, :], in0=mm2[b][:, :],
                        scalar1=aall[:, b2c:b2c + 1])
            else:
                yst = fpool.tile([128, S], F16, tag="yst")
                nc.vector.tensor_scalar_add(out=yst[:, :], in0=mm2[:, :],
                                            scalar1=aall[:, b2c:b2c + 1])
            st[(l, 'yst')] = yst

        def t_y2(l):
            Co = COUT[l]
            yst = st[(l, 'yst')]
            if l == 3:
                y2q = work.tile([128, S], F16, tag="y2q")
                y2k = work.tile([128, S], F16, tag="y2k")
                nc.vector.tensor_mul(out=y2q, in0=yst[:, 0, :], in1=yst[:, 0, :])
                nc.vector.tensor_mul(out=y2k, in0=yst[:, 1, :], in1=yst[:, 1, :])
                st[(l, 'y2')] = (y2q, y2k)
            else:
                nc.vector.tensor_mul(out=y2st[l][0:Co, :], in0=yst[0:Co, :],
                                     in1=yst[0:Co, :])
                nc.vector.tensor_mul(out=y2st[l][64:64 + Co, :],
                                     in0=yst[64:64 + Co, :],
                                     in1=yst[64:64 + Co, :])

        def t_ssq(l):
            if l == 3:
                y2q, y2k = st[(l, 'y2')]
                ssq_q = psm.tile([1, S], F32, tag="sm", name="ssq3q")
                ssq_k = psm.tile([1, S], F32, tag="sm", name="ssq3k")
                nc.tensor.matmul(ssq_q, lhsT=hall[:, 0:1], rhs=y2q,
                                 start=True, stop=True)
                nc.tensor.matmul(ssq_k, lhsT=hall[:, 0:1], rhs=y2k,
                                 start=True, stop=True)
                st[(l, 'ssq')] = (ssq_q, ssq_k)
            else:
                ssq = psm.tile([2, S], F32, tag="sm", name=f"ssq{l}")
                sc = SEL2[l]
                nc.tensor.matmul(ssq, lhsT=hall[:, sc:sc + 2], rhs=y2st[l],
                                 start=True, stop=True)
                st[(l, 'ssq')] = ssq

        def t_ln(l):
            if l == 3:
                ssq_q, ssq_k = st[(l, 'ssq')]
                t3q = work.tile([1, S], F32, tag="t3q")
                t3k = work.tile([1, S], F32, tag="t3k")
                nc.scalar.activation(out=t3q, in_=ssq_q, func=AF.Ln)
                nc.scalar.activation(out=t3k, in_=ssq_k, func=AF.Ln)
                st[(l, 't1')] = (t3q, t3k)
            else:
                t1 = work.tile([2, S], F32, tag="t1")
                nc.scalar.activation(out=t1, in_=st[(l, 'ssq')], func=AF.Ln)
                st[(l, 't1')] = t1

        def t_rn(l):
            if l == 3:
                t3q, t3k = st[(l, 't1')]
                rnq = work.tile([1, S], F16, tag="rnq")
                rnk = work.tile([1, S], F16, tag="rnk")
                nc.scalar.activation(out=rnq, in_=t3q, func=AF.Exp,
                                     scale=-0.5, bias=aall[0:1, RNC:RNC + 1])
                nc.scalar.activation(out=rnk, in_=t3k, func=AF.Exp,
                                     scale=-0.5, bias=aall[0:1, RNC:RNC + 1])
                st[(l, 'rn')] = (rnq, rnk)
            else:
                rn = work.tile([2, S], F16, tag="rn")
                nc.scalar.activation(out=rn, in_=st[(l, 't1')], func=AF.Exp,
                                     scale=-0.5, bias=aall[0:2, RNC:RNC + 1])
                st[(l, 'rn')] = rn

        def t_bc(l):
            if l == 3:
                rnq, rnk = st[(l, 'rn')]
                bcq = psm.tile([128, S], F32, tag="sm", name="bc3q")
                bck = psm.tile([128, S], F32, tag="sm", name="bc3k")
                nc.tensor.matmul(bcq, lhsT=hrow[0:1, 0:128], rhs=rnq,
                                 start=True, stop=True)
                nc.tensor.matmul(bck, lhsT=hrow[0:1, 0:128], rhs=rnk,
                                 start=True, stop=True)
                st[(l, 'bc')] = (bcq, bck)
            else:
                bc = psm.tile([128, S], F32, tag="sm", name=f"bc{l}")
                bs = BSEL[l]
                nc.tensor.matmul(bc, lhsT=hrow[0:2, bs:bs + 128],
                                 rhs=st[(l, 'rn')], start=True, stop=True)
                st[(l, 'bc')] = bc

        def t_f(l):
            Co = COUT[l]
            yst = st[(l, 'yst')]
            fq = fpool.tile([128, S], F16, tag="fq")
            fk = fpool.tile([128, S], F16, tag="fk")
            if l == 3:
                bcq, bck = st[(l, 'bc')]
                nc.vector.tensor_mul(out=fq, in0=yst[:, 0, :], in1=bcq)
                nc.vector.tensor_mul(out=fk, in0=yst[:, 1, :], in1=bck)
            else:
                bc = st[(l, 'bc')]
                nc.vector.tensor_mul(out=fq[0:Co, :], in0=yst[0:Co, :],
                                     in1=bc[0:Co, :])
                nc.vector.tensor_mul(out=fk[0:Co, :], in0=yst[64:64 + Co, :],
                                     in1=bc[64:64 + Co, :])
            st[(l, 'f')] = (fq, fk)

        def t_pos(l):
            Co = COUT[l]
            fq, fk = st[(l, 'f')]
            pprod = work.tile([128, S], F16, tag="pp")
            nc.vector.tensor_mul(out=pprod[0:Co, :], in0=fq[0:Co, :],
                                 in1=fk[0:Co, :])
            jp = work.tile([128, S], F16, tag="jp")
            nc.vector.tensor_scalar(
                out=jp[0:Co, :], in0=pprod[0:Co, :], scalar1=1.0, scalar2=0.0,
                op0=ALU.mult, op1=ALU.add,
                accum_out=catL[0:Co, 16 + l:17 + l])

        def g_gram(l, m):
            Co = COUT[l]
            fq, fk = st[(l, 'f')]
            g = pg.tile([128, S], F32, tag="g", name=f"g{l}_{m}")
            nc.tensor.matmul(g, lhsT=fq[0:Co, m * 128:(m + 1) * 128],
                             rhs=fk[0:Co, :], start=True, stop=True)
            st[(l, 'g', m)] = g

        def g_exp(l, m):
            # exp in place on the PSUM tile; the row sum (over t) goes
            # straight to ZD via the ACT accumulator
            g = st.pop((l, 'g', m))
            i = 4 * l + m
            nc.scalar.activation(out=g, in_=g, func=AF.Exp, scale=1.0,
                                 bias=aall[:, BEC:BEC + 1],
                                 accum_out=ZD[:, i:i + 1])

        # ---------------- schedule ----------------
        emit_subs(1)
        emit_mlp1(1)
        emit_mlp2(1)
        emit_subs(3, branches=(0,))
        t_evict(1); t_y2(1); t_ssq(1); t_ln(1); t_rn(1); t_bc(1); t_f(1)
        emit_mlp1(3, branches=(0,))
        t_pos(1)
        for m in range(4):
            g_gram(1, m)
            g_exp(1, m)
        emit_subs(3, branches=(1,))
        emit_mlp1(3, branches=(1,))
        emit_subs(2)
        emit_mlp1(2)
        emit_mlp2(3)
        emit_mlp2(2)
        emit_subs(0)
        emit_mlp1(0)
        emit_mlp2(0)
        # interleave the three remaining tails micro-phase round-robin
        for ph in (t_evict, t_y2, t_ssq, t_ln, t_rn, t_bc, t_f):
            for l in TRIO:
                ph(l)
        for m in range(4):
            for l in TRIO:
                g_gram(l, m)
                g_exp(l, m)
        for l in TRIO:
            t_pos(l)

        # finale: res = sum ln ZD - sum l_pos' + COMB_C
        nc.scalar.activation(out=catL[:, 0:16], in_=ZD[:, :], func=AF.Ln)
        pmmf = psm.tile([1, 20], F32, tag="sm", name="pmmf")
        nc.tensor.matmul(pmmf, lhsT=aall[:, OCOL:OCOL + 1], rhs=catL,
                         start=True, stop=True)
        wprod = const.tile([1, 20], F32)
        nc.vector.tensor_mul(out=wprod, in0=pmmf,
                             in1=aall[0:1, WVC:WVC + 20])
        r0 = const.tile([1, 1], F32)
        nc.vector.reduce_sum(out=r0, in_=wprod, axis=mybir.AxisListType.X)
        res = const.tile([1, 1], F32)
        nc.vector.tensor_scalar(out=res, in0=r0, scalar1=1.0,
                                scalar2=COMB_C, op0=ALU.mult, op1=ALU.add)
        nc.sync.dma_start(out=out[:, :], in_=res[:, :])
    nc.finalize()
    return nc


_NC_CACHE = {}


def _get_nc(key="v4", **kw):
    if key not in _NC_CACHE:
        _NC_CACHE[key] = _build_nc(**kw)
    return _NC_CACHE[key]


def _host_blobs(inputs):
    nidx, cidx = [], []
    for l in range(4):
        sid = np.asarray(inputs[f"sid{l}"]).astype(np.int64)
        nidx.append(((sid[:, 0:1] + _DH) * 32 + (sid[:, 1:2] + _DW)).reshape(-1))
        cidx.append((sid[:, 0] + 1) * 32 + (sid[:, 1] + 1))

    wts = np.zeros((128, WTOT), dtype=np.float16)
    aux = np.zeros((128, AUXW), dtype=np.float32)
    for l in range(4):
        w1T = np.asarray(inputs[f"w1_{l}"]).astype(np.float32).T
        w2T = np.asarray(inputs[f"w2_{l}"]).astype(np.float32).T
        b1 = np.asarray(inputs[f"b1_{l}"]).astype(np.float32)
        b2 = np.asarray(inputs[f"b2_{l}"]).astype(np.float32)
        C, Co = CS[l], COUT[l]
        for k in range(KC[l]):
            rows = min(128, C - k * 128)
            c0 = W1C[(l, k)]
            wts[0:rows, c0:c0 + C] = w1T[k * 128:k * 128 + rows, :]
            c0 = W2C[(l, k)]
            wts[0:rows, c0:c0 + Co] = w2T[k * 128:k * 128 + rows, :]
        for m in range(KC[l]):
            rows = min(128, C - m * 128)
            aux[0:rows, B1C[(l, m)]] = b1[m * 128:m * 128 + rows]
        aux[0:Co, B2C[l]] = b2
        if l != 3:
            aux[64:64 + Co, B2C[l]] = b2
    aux[:, OCOL] = 1.0
    aux[:, RNC] = BIAS_RN
    aux[:, BEC] = BIAS_E
    aux[0, WVC:WVC + 16] = 1.0
    aux[0, WVC + 16:WVC + 20] = -1.0

    auxh = np.zeros((128, 8), dtype=np.float16)
    auxh[:, 0] = 1.0
    for l in range(3):
        Co = COUT[l]
        auxh[0:Co, SEL2[l]] = 1.0
        auxh[64:64 + Co, SEL2[l] + 1] = 1.0
    auxh2 = np.zeros((2, 512), dtype=np.float16)
    auxh2[0, 0:128] = 1.0
    for l in range(3):
        Co = COUT[l]
        auxh2[0, BSEL[l]:BSEL[l] + Co] = 1.0
        auxh2[1, BSEL[l] + 64:BSEL[l] + 64 + Co] = 1.0

    # partition-major x blobs: [128, NCH*576] fp8, chunk n at cols n*576,
    # [neigh 512 | center 64] per chunk
    f8 = mybir.dt.np(F8)
    xqs = [np.zeros((128, NCH * 576), dtype=f8) for _ in range(NCORES)]
    xks = [np.zeros((128, NCH * 576), dtype=f8) for _ in range(NCORES)]
    for l in range(4):
        C = CS[l]
        fq = np.asarray(inputs[f"fq{l}"])[:, :, :32, :32].reshape(NCORES, C, 1024)
        fk = np.asarray(inputs[f"fk{l}"])[:, :, :32, :32].reshape(NCORES, C, 1024)
        qn = fq[:, :, nidx[l]].astype(f8)
        qc = fq[:, :, cidx[l]].astype(f8)
        kn = fk[:, :, nidx[l]].astype(f8)
        kc_ = fk[:, :, cidx[l]].astype(f8)
        for b in range(NCORES):
            for k in range(KC[l]):
                n0 = CH[(l, k)] * 576
                rows = min(128, C - k * 128)
                sl = slice(k * 128, k * 128 + rows)
                xqs[b][0:rows, n0:n0 + 512] = qn[b, sl, :]
                xqs[b][0:rows, n0 + 512:n0 + 576] = qc[b, sl, :]
                xks[b][0:rows, n0:n0 + 512] = kn[b, sl, :]
                xks[b][0:rows, n0 + 512:n0 + 576] = kc_[b, sl, :]
    return wts, aux, auxh, auxh2, xqs, xks


_LAST_RESULT = {}


def kernel(**inputs):
    assert int(inputs.get("start_layer", 0)) == 0
    assert int(inputs.get("end_layer", 4)) == 4
    assert int(inputs.get("num_s", 64)) == 64

    nc = _get_nc()
    wts, aux, auxh, auxh2, xqs, xks = _host_blobs(inputs)
    in_maps = [
        {"xq": xqs[b], "xk": xks[b], "wts": wts, "aux": aux,
         "auxh": auxh, "auxh2": auxh2}
        for b in range(NCORES)
    ]
    r = run_bass_kernel_spmd(nc, in_maps, core_ids=list(range(NCORES)))
    _LAST_RESULT["r"] = r
    partials = [np.float64(r.results[b]["out"][0, 0]) for b in range(NCORES)]
    loss = np.float32(sum(partials) / (NCORES * S))
    return np.asarray(loss, dtype=np.float32)


# revision 14
# speedup vs baseline: 1.6198x; 1.0570x over previous
"""CCPL contrastive loss kernel for Trainium2 (8 NeuronCores, SPMD data-parallel).

Contract: kernel(**inputs) takes FULL unsharded inputs, returns the FULL scalar
loss (float32, shape ()).  Host does indexing-only gathers (sid in [0,30) means
only the top-left 32x32 corner of every feature map is read); core b processes
batch b end-to-end on device; host sums the 8 partial CE sums / (8*512).

v4 design (vs 64.2us baseline):
 - x uploaded partition-major [128, NCH*1024] with the center duplicated 8x:
   few large DMA descriptors, and x = neigh - center is a flat fp16 SBUF sub
   (DVE 2x mode); x split across the gpsimd + scalar DGE queues
 - f' = y * rn with rn = exp(-0.5 ln ssq - 0.5 ln tau): both branches carry
   1/sqrt(tau) so the Gram G' = fq'^T fk' is already G/tau
 - Gram computed transposed (G'[t-block, s]) as 4 single-bank PSUM tiles per
   layer from a 2-slot ring; exp per tile (ACT, bias -0.5/tau keeps E' fp16);
   Z row-sums as ones^T E' PE matmuls accumulating over the 4 t-blocks
 - sum_s ln Z via ACT Ln with accum_out; sum_s l_pos via fp16 pprod + DVE
   tensor_scalar accum_out (no big DVE reduces anywhere)
 - l0-2: q/k band-stacked in partitions [0:Co]/[64:64+Co] for mm2/ssq/bc
   (selector matmuls); l3 mm2 as two 1-bank tiles
 - PSUM = mm1 ring (2x1) + small ring (4x1) + gram ring (2x1) = 8 banks
 - tails of the last three layers emitted micro-phase round-robin so their
   serial dependency chains overlap each other and the remaining MLP work
"""

import numpy as np
from contextlib import ExitStack

import concourse.bass as bass
import concourse.bacc as bacc
import concourse.tile as tile
from concourse import mybir
from concourse.bass_utils import run_bass_kernel_spmd

F32 = mybir.dt.float32
F16 = mybir.dt.float16
F8 = mybir.dt.float8e4
ALU = mybir.AluOpType
AF = mybir.ActivationFunctionType

# Force Exp/Ln/Relu into the one table set containing all three so the kernel
# pays a single ACT_TABLE_LOAD.
_COMBINED_SET = "natural_log_exp_and_others"
_orig_get_tables = bacc.get_activation_tables


def _patched_get_tables(arch):
    t = _orig_get_tables(arch)
    return {
        name: (fns if name == _COMBINED_SET else set())
        for name, fns in t.items()
    }


bacc.get_activation_tables = _patched_get_tables

TAU = 0.07
NCORES = 8
S = 512
CS = [64, 128, 256, 512]
COUT = [16, 32, 64, 128]
KC = [1, 1, 2, 4]
NCH = sum(KC)
LORD = (1, 3, 2, 0)          # processing order (also x/wts blob order)
TRIO = (3, 2, 0)             # tail-interleaved layers
_DH = np.array([0, 0, 0, 1, 1, 2, 2, 2], dtype=np.int64)
_DW = np.array([0, 1, 2, 0, 2, 0, 1, 2], dtype=np.int64)
BIAS_RN = -0.5 * float(np.log(TAU))   # rn' = exp(-0.5 ln ssq + BIAS_RN)
BIAS_E = -0.5 / TAU                   # exp(G' - 0.5/tau)
COMB_C = 4 * S * 0.5 / TAU            # restores the BIAS_E shift in sum_s lnZ

# chunk indices in LORD order ------------------------------------------------
CH = {}
_c = 0
for _l in LORD:
    for _k in range(KC[_l]):
        CH[(_l, _k)] = _c
        _c += 1

# weight blob column offsets, LORD order so DMA slices are contiguous
# w1 lives in the fp8 blob (DoubleRow matmuls), w2 in the fp16 blob
W1C, W2C, WBLK, W8BLK = {}, {}, {}, {}
_c = 0
_c8 = 0
for _l in LORD:
    _s, _s8 = _c, _c8
    for _k in range(KC[_l]):
        W1C[(_l, _k)] = _c8
        _c8 += CS[_l]
    for _k in range(KC[_l]):
        W2C[(_l, _k)] = _c
        _c += COUT[_l]
    WBLK[_l] = (_s, _c)
    W8BLK[_l] = (_s8, _c8)
WTOT = _c
W8TOT = _c8

# aux blob (f32): cols 0..7 b1 chunks, 8..11 b2 (band-stacked for l<3), 12 ones
B1C = {}
_c = 0
for _l in range(4):
    for _m in range(KC[_l]):
        B1C[(_l, _m)] = _c
        _c += 1
B2C = {l: 8 + l for l in range(4)}
OCOL = 12
RNC = 13          # BIAS_RN column
BEC = 14          # BIAS_E column
WVC = 16          # [1]*16 | [-1]*4 final-combine weights (row 0)
AUXW = 36
# auxh (f16) [128, 8]: col 0 ones column; cols 1+2l / 2+2l: ssq band selectors
# auxh2 (f16) [2, 512]: cols 0:128 ones row (row 0); bsel_l at 128+128l
SEL2 = {l: 1 + 2 * l for l in range(3)}
BSEL = {l: 128 + 128 * l for l in range(3)}

# engine assignment knobs ----------------------------------------------------
RELU_ENG = {}
for _l in range(4):
    for _m in range(KC[_l]):
        for _b in range(2):
            RELU_ENG[(_l, _m, _b)] = 'a' if (_m + _b) % 2 == 0 else 'v'
EVICT_ENG = {0: 'v', 1: 'v', 2: 'v', 3: 'v'}


def _build_nc(relu_eng=None, evict_eng=None):
    relu_eng = relu_eng or RELU_ENG
    evict_eng = evict_eng or EVICT_ENG
    nc = bacc.Bacc()
    xq = nc.dram_tensor("xq", [128, NCH * 576], F8, kind="ExternalInput")
    xk = nc.dram_tensor("xk", [128, NCH * 576], F8, kind="ExternalInput")
    wts = nc.dram_tensor("wts", [128, WTOT], F16, kind="ExternalInput")
    wts8 = nc.dram_tensor("wts8", [128, W8TOT], F8, kind="ExternalInput")
    aux = nc.dram_tensor("aux", [128, AUXW], F32, kind="ExternalInput")
    auxh = nc.dram_tensor("auxh", [128, 8], F16, kind="ExternalInput")
    auxh2 = nc.dram_tensor("auxh2", [2, 512], F16, kind="ExternalInput")
    out = nc.dram_tensor("out", [1, 1], F32, kind="ExternalOutput")

    with ExitStack() as ctx:
        tc = ctx.enter_context(tile.TileContext(nc))
        const = ctx.enter_context(tc.tile_pool(name="const", bufs=1))
        work = ctx.enter_context(tc.tile_pool(name="work", bufs=3))
        fpool = ctx.enter_context(tc.tile_pool(name="fpool", bufs=3))
        # PSUM: mm1 (2x1 bank) + sm (4x1 bank) + g (2x1 bank) = 8 banks
        pmm = ctx.enter_context(tc.tile_pool(name="pmm", bufs=2, space="PSUM"))
        psm = ctx.enter_context(tc.tile_pool(name="psm", bufs=4, space="PSUM"))
        pg = ctx.enter_context(tc.tile_pool(name="pg", bufs=2, space="PSUM"))

        xq_s = const.tile([128, NCH, 576], F8)
        xk_s = const.tile([128, NCH, 576], F8)
        wall = const.tile([128, WTOT], F16)
        w8all = const.tile([128, W8TOT], F8)
        aall = const.tile([128, AUXW], F32)
        hall = const.tile([128, 8], F16)
        hrow = const.tile([2, 512], F16)
        xall = (xq_s, xk_s)
        xdr = (xq, xk)

        def xdma(eng, b, c0, c1):
            eng.dma_start(out=xall[b][:, c0:c1, :],
                          in_=xdr[b][:, c0 * 576:c1 * 576])

        # l1 + l3q + first half of l3k on gpsimd; rest of x on scalar
        c1, _ = CH[(1, 0)], None
        c3 = CH[(3, 0)]
        c2 = CH[(2, 0)]
        c0_ = CH[(0, 0)]
        xdma(nc.gpsimd, 0, c1, c1 + 1)
        xdma(nc.gpsimd, 1, c1, c1 + 1)
        xdma(nc.gpsimd, 0, c3, c3 + 2)
        xdma(nc.gpsimd, 1, c3, c3 + 2)
        xdma(nc.gpsimd, 0, c0_, c0_ + 1)
        xdma(nc.gpsimd, 1, c0_, c0_ + 1)
        xdma(nc.scalar, 0, c3 + 2, c3 + 4)
        xdma(nc.scalar, 1, c3 + 2, c3 + 4)
        xdma(nc.scalar, 0, c2, c2 + 2)
        xdma(nc.scalar, 1, c2, c2 + 2)
        w0, w1_ = W8BLK[LORD[0]]
        nc.sync.dma_start(out=w8all[:, w0:w1_], in_=wts8[:, w0:w1_])
        nc.sync.dma_start(out=hall, in_=auxh[:, :])
        nc.sync.dma_start(out=hrow, in_=auxh2[:, :])
        nc.sync.dma_start(out=aall, in_=aux[:, :])
        for l in LORD[1:]:
            w0, w1_ = W8BLK[l]
            nc.sync.dma_start(out=w8all[:, w0:w1_], in_=wts8[:, w0:w1_])
        nc.sync.dma_start(out=wall, in_=wts[:, :])

        # shared state
        xsub = [const.tile([128, NCH, S], F8, tag=f"xsub{b}",
                           name=f"xsub{b}") for b in range(2)]
        # per-layer band-stacked y^2 tiles (pre-zeroed; sharing one tile
        # across layers corrupts a layer's ssq when tails interleave)
        y2st = {l: const.tile([128, S], F16, tag=f"y2st{l}", name=f"y2st{l}")
                for l in (0, 1, 2)}
        ZD = const.tile([128, 16], F32)        # Z per (s-block, layer*4+m)
        # cols 0..15: ln(ZD); cols 16..19: per-layer banded sum_s l_pos'
        catL = const.tile([128, 20], F32)
        for l in (0, 1, 2):
            nc.gpsimd.memset(y2st[l][:, :], 0.0)
        nc.gpsimd.memset(catL[:, :], 0.0)

        st = {}   # per-layer tiles carried between phases

        def emit_subs(l, branches=(0, 1)):
            a0, a1 = CH[(l, 0)], CH[(l, 0)] + KC[l]
            for b in branches:
                in0 = xall[b][:, a0:a1, 0:512].rearrange(
                    "p n (s j) -> p n s j", j=8)
                cb = xall[b][:, a0:a1, 512:576]
                in1 = bass.AP(cb.tensor, cb.offset, [*cb.ap, [0, 8]])
                nc.vector.tensor_sub(
                    out=xsub[b][:, a0:a1, :].rearrange(
                        "p n (s j) -> p n s j", j=8),
                    in0=in0, in1=in1)

        def emit_mlp1(l, branches=(0, 1)):
            C, Co, K = CS[l], COUT[l], KC[l]
            if (l, 'h') not in st:
                st[(l, 'h')] = const.tile([128, K, 2, S], F16, tag=f"h{l}",
                                          name=f"h{l}")
            h = st[(l, 'h')]
            for b in branches:
                for m in range(K):
                    rows = min(128, C - m * 128)
                    mm1 = pmm.tile([128, S], F32, tag="mm1")
                    if K >= 2:
                        # fp8 DoubleRow: contract chunk pairs (k, k+1) in one
                        # matmul at 0.5 cycles/row
                        for k in range(0, K, 2):
                            c0 = W1C[(l, k)]
                            wpair = w8all[:, c0:c0 + 2 * C].rearrange(
                                "p (i r) -> p i r", i=2)[:, :,
                                                         m * 128:m * 128 + rows]
                            nc.tensor.matmul(
                                mm1[0:rows, :],
                                lhsT=wpair,
                                rhs=xsub[b][:, CH[(l, k)]:CH[(l, k)] + 2, :],
                                start=(k == 0), stop=(k == K - 2),
                                perf_mode=mybir.MatmulPerfMode.DoubleRow)
                    else:
                        c0 = W1C[(l, 0)] + m * 128
                        nc.tensor.matmul(
                            mm1[0:rows, :],
                            lhsT=w8all[:, c0:c0 + rows],
                            rhs=xsub[b][:, CH[(l, 0)], :],
                            start=True, stop=True)
                    bc1 = B1C[(l, m)]
                    if relu_eng[(l, m, b)] == 'a':
                        nc.scalar.activation(
                            out=h[0:rows, m, b, :], in_=mm1[0:rows, :],
                            func=AF.Relu, bias=aall[0:rows, bc1:bc1 + 1],
                            scale=1.0)
                    else:
                        nc.vector.tensor_scalar(
                            out=h[0:rows, m, b, :], in0=mm1[0:rows, :],
                            scalar1=aall[0:rows, bc1:bc1 + 1], scalar2=0.0,
                            op0=ALU.add, op1=ALU.max)

        def emit_mlp2(l):
            C, Co, K = CS[l], COUT[l], KC[l]
            h = st[(l, 'h')]
            if l == 3:
                mm2 = [psm.tile([128, S], F32, tag="sm", name=f"mm2q{l}"),
                       psm.tile([128, S], F32, tag="sm", name=f"mm2k{l}")]
                for b in range(2):
                    for k in range(K):
                        c0 = W2C[(l, k)]
                        nc.tensor.matmul(
                            mm2[b][:, :],
                            lhsT=wall[0:128, c0:c0 + Co],
                            rhs=h[:, k, b, :],
                            start=(k == 0), stop=(k == K - 1))
            else:
                mm2 = psm.tile([128, S], F32, tag="sm", name=f"mm2st{l}")
                for b in range(2):
                    for k in range(K):
                        rows = min(128, C - k * 128)
                        c0 = W2C[(l, k)]
                        nc.tensor.matmul(
                            mm2[64 * b:64 * b + Co, :],
                            lhsT=wall[0:rows, c0:c0 + Co],
                            rhs=h[0:rows, k, b, :],
                            start=(k == 0), stop=(k == K - 1))
            st[(l, 'mm2')] = mm2

        def _ts_add(eng, outt, int_, bias_ap):
            if eng == 'a':
                nc.scalar.activation(out=outt, in_=int_, func=AF.Relu,
                                     bias=bias_ap, scale=1.0)  # unused
            else:
                nc.vector.tensor_scalar_add(out=outt, in0=int_,
                                            scalar1=bias_ap)

        # ---- tail micro-phases (t_*: evict .. pos, g_*: gram/exp/Z/lnZ) ----
        def t_evict(l):
            Co = COUT[l]
            mm2 = st[(l, 'mm2')]
            b2c = B2C[l]
            if l == 3:
                yst = fpool.tile([128, 2, S], F16, tag="yst3")
                for b in range(2):
                    nc.vector.tensor_scalar_add(
                        out=yst[:, b, :], in0=mm2[b][:, :],
                        scalar1=aall[:, b2c:b2c + 1])
            else:
                yst = fpool.tile([128, S], F16, tag="yst")
                nc.vector.tensor_scalar_add(out=yst[:, :], in0=mm2[:, :],
                                            scalar1=aall[:, b2c:b2c + 1])
            st[(l, 'yst')] = yst

        def t_y2(l):
            Co = COUT[l]
            yst = st[(l, 'yst')]
            if l == 3:
                y2q = work.tile([128, S], F16, tag="y2q")
                y2k = work.tile([128, S], F16, tag="y2k")
                nc.vector.tensor_mul(out=y2q, in0=yst[:, 0, :], in1=yst[:, 0, :])
                nc.vector.tensor_mul(out=y2k, in0=yst[:, 1, :], in1=yst[:, 1, :])
                st[(l, 'y2')] = (y2q, y2k)
            else:
                nc.vector.tensor_mul(out=y2st[l][0:Co, :], in0=yst[0:Co, :],
                                     in1=yst[0:Co, :])
                nc.vector.tensor_mul(out=y2st[l][64:64 + Co, :],
                                     in0=yst[64:64 + Co, :],
                                     in1=yst[64:64 + Co, :])

        def t_ssq(l):
            if l == 3:
                y2q, y2k = st[(l, 'y2')]
                ssq_q = psm.tile([1, S], F32, tag="sm", name="ssq3q")
                ssq_k = psm.tile([1, S], F32, tag="sm", name="ssq3k")
                nc.tensor.matmul(ssq_q, lhsT=hall[:, 0:1], rhs=y2q,
                                 start=True, stop=True)
                nc.tensor.matmul(ssq_k, lhsT=hall[:, 0:1], rhs=y2k,
                                 start=True, stop=True)
                st[(l, 'ssq')] = (ssq_q, ssq_k)
            else:
                ssq = psm.tile([2, S], F32, tag="sm", name=f"ssq{l}")
                sc = SEL2[l]
                nc.tensor.matmul(ssq, lhsT=hall[:, sc:sc + 2], rhs=y2st[l],
                                 start=True, stop=True)
                st[(l, 'ssq')] = ssq

        def t_ln(l):
            if l == 3:
                ssq_q, ssq_k = st[(l, 'ssq')]
                t3q = work.tile([1, S], F32, tag="t3q")
                t3k = work.tile([1, S], F32, tag="t3k")
                nc.scalar.activation(out=t3q, in_=ssq_q, func=AF.Ln)
                nc.scalar.activation(out=t3k, in_=ssq_k, func=AF.Ln)
                st[(l, 't1')] = (t3q, t3k)
            else:
                t1 = work.tile([2, S], F32, tag="t1")
                nc.scalar.activation(out=t1, in_=st[(l, 'ssq')], func=AF.Ln)
                st[(l, 't1')] = t1

        def t_rn(l):
            if l == 3:
                t3q, t3k = st[(l, 't1')]
                rnq = work.tile([1, S], F16, tag="rnq")
                rnk = work.tile([1, S], F16, tag="rnk")
                nc.scalar.activation(out=rnq, in_=t3q, func=AF.Exp,
                                     scale=-0.5, bias=aall[0:1, RNC:RNC + 1])
                nc.scalar.activation(out=rnk, in_=t3k, func=AF.Exp,
                                     scale=-0.5, bias=aall[0:1, RNC:RNC + 1])
                st[(l, 'rn')] = (rnq, rnk)
            else:
                rn = work.tile([2, S], F16, tag="rn")
                nc.scalar.activation(out=rn, in_=st[(l, 't1')], func=AF.Exp,
                                     scale=-0.5, bias=aall[0:2, RNC:RNC + 1])
                st[(l, 'rn')] = rn

        def t_bc(l):
            if l == 3:
                rnq, rnk = st[(l, 'rn')]
                bcq = psm.tile([128, S], F32, tag="sm", name="bc3q")
                bck = psm.tile([128, S], F32, tag="sm", name="bc3k")
                nc.tensor.matmul(bcq, lhsT=hrow[0:1, 0:128], rhs=rnq,
                                 start=True, stop=True)
                nc.tensor.matmul(bck, lhsT=hrow[0:1, 0:128], rhs=rnk,
                                 start=True, stop=True)
                st[(l, 'bc')] = (bcq, bck)
            else:
                bc = psm.tile([128, S], F32, tag="sm", name=f"bc{l}")
                bs = BSEL[l]
                nc.tensor.matmul(bc, lhsT=hrow[0:2, bs:bs + 128],
                                 rhs=st[(l, 'rn')], start=True, stop=True)
                st[(l, 'bc')] = bc

        def t_f(l):
            Co = COUT[l]
            yst = st[(l, 'yst')]
            fq = fpool.tile([128, S], F16, tag="fq")
            fk = fpool.tile([128, S], F16, tag="fk")
            if l == 3:
                bcq, bck = st[(l, 'bc')]
                nc.vector.tensor_mul(out=fq, in0=yst[:, 0, :], in1=bcq)
                nc.vector.tensor_mul(out=fk, in0=yst[:, 1, :], in1=bck)
            else:
                bc = st[(l, 'bc')]
                nc.vector.tensor_mul(out=fq[0:Co, :], in0=yst[0:Co, :],
                                     in1=bc[0:Co, :])
                nc.vector.tensor_mul(out=fk[0:Co, :], in0=yst[64:64 + Co, :],
                                     in1=bc[64:64 + Co, :])
            st[(l, 'f')] = (fq, fk)

        def t_pos(l):
            Co = COUT[l]
            fq, fk = st[(l, 'f')]
            pprod = work.tile([128, S], F16, tag="pp")
            nc.vector.tensor_mul(out=pprod[0:Co, :], in0=fq[0:Co, :],
                                 in1=fk[0:Co, :])
            jp = work.tile([128, S], F16, tag="jp")
            nc.vector.tensor_scalar(
                out=jp[0:Co, :], in0=pprod[0:Co, :], scalar1=1.0, scalar2=0.0,
                op0=ALU.mult, op1=ALU.add,
                accum_out=catL[0:Co, 16 + l:17 + l])

        def g_gram(l, m):
            Co = COUT[l]
            fq, fk = st[(l, 'f')]
            g = pg.tile([128, S], F32, tag="g", name=f"g{l}_{m}")
            nc.tensor.matmul(g, lhsT=fq[0:Co, m * 128:(m + 1) * 128],
                             rhs=fk[0:Co, :], start=True, stop=True)
            st[(l, 'g', m)] = g

        def g_exp(l, m):
            # exp in place on the PSUM tile; the row sum (over t) goes
            # straight to ZD via the ACT accumulator
            g = st.pop((l, 'g', m))
            i = 4 * l + m
            nc.scalar.activation(out=g, in_=g, func=AF.Exp, scale=1.0,
                                 bias=aall[:, BEC:BEC + 1],
                                 accum_out=ZD[:, i:i + 1])

        # ---------------- schedule ----------------
        emit_subs(1)
        emit_mlp1(1)
        emit_mlp2(1)
        emit_subs(3, branches=(0,))
        t_evict(1); t_y2(1); t_ssq(1); t_ln(1); t_rn(1); t_bc(1); t_f(1)
        emit_mlp1(3, branches=(0,))
        t_pos(1)
        for m in range(4):
            g_gram(1, m)
            g_exp(1, m)
        emit_subs(3, branches=(1,))
        emit_mlp1(3, branches=(1,))
        emit_subs(2)
        emit_mlp1(2)
        emit_mlp2(3)
        emit_mlp2(2)
        emit_subs(0)
        emit_mlp1(0)
        emit_mlp2(0)
        # interleave the three remaining tails micro-phase round-robin
        for ph in (t_evict, t_y2, t_ssq, t_ln, t_rn, t_bc, t_f):
            for l in TRIO:
                ph(l)
        for m in range(4):
            for l in TRIO:
                g_gram(l, m)
                g_exp(l, m)
        for l in TRIO:
            t_pos(l)

        # finale: res = sum ln ZD - sum l_pos' + COMB_C
        nc.scalar.activation(out=catL[:, 0:16], in_=ZD[:, :], func=AF.Ln)
        pmmf = psm.tile([1, 20], F32, tag="sm", name="pmmf")
        nc.tensor.matmul(pmmf, lhsT=aall[:, OCOL:OCOL + 1], rhs=catL,
                         start=True, stop=True)
        wprod = const.tile([1, 20], F32)
        nc.vector.tensor_mul(out=wprod, in0=pmmf,
                             in1=aall[0:1, WVC:WVC + 20])
        r0 = const.tile([1, 1], F32)
        nc.vector.reduce_sum(out=r0, in_=wprod, axis=mybir.AxisListType.X)
        res = const.tile([1, 1], F32)
        nc.vector.tensor_scalar(out=res, in0=r0, scalar1=1.0,
                                scalar2=COMB_C, op0=ALU.mult, op1=ALU.add)
        nc.sync.dma_start(out=out[:, :], in_=res[:, :])
    nc.finalize()
    return nc


_NC_CACHE = {}


def _get_nc(key="v4", **kw):
    if key not in _NC_CACHE:
        _NC_CACHE[key] = _build_nc(**kw)
    return _NC_CACHE[key]


def _host_blobs(inputs):
    nidx, cidx = [], []
    for l in range(4):
        sid = np.asarray(inputs[f"sid{l}"]).astype(np.int64)
        nidx.append(((sid[:, 0:1] + _DH) * 32 + (sid[:, 1:2] + _DW)).reshape(-1))
        cidx.append((sid[:, 0] + 1) * 32 + (sid[:, 1] + 1))

    f8w = mybir.dt.np(F8)
    wts = np.zeros((128, WTOT), dtype=np.float16)
    wts8 = np.zeros((128, W8TOT), dtype=f8w)
    aux = np.zeros((128, AUXW), dtype=np.float32)
    for l in range(4):
        w1T = np.asarray(inputs[f"w1_{l}"]).astype(np.float32).T
        w2T = np.asarray(inputs[f"w2_{l}"]).astype(np.float32).T
        b1 = np.asarray(inputs[f"b1_{l}"]).astype(np.float32)
        b2 = np.asarray(inputs[f"b2_{l}"]).astype(np.float32)
        C, Co = CS[l], COUT[l]
        for k in range(KC[l]):
            rows = min(128, C - k * 128)
            c0 = W1C[(l, k)]
            wts8[0:rows, c0:c0 + C] = w1T[k * 128:k * 128 + rows, :].astype(f8w)
            c0 = W2C[(l, k)]
            wts[0:rows, c0:c0 + Co] = w2T[k * 128:k * 128 + rows, :]
        for m in range(KC[l]):
            rows = min(128, C - m * 128)
            aux[0:rows, B1C[(l, m)]] = b1[m * 128:m * 128 + rows]
        aux[0:Co, B2C[l]] = b2
        if l != 3:
            aux[64:64 + Co, B2C[l]] = b2
    aux[:, OCOL] = 1.0
    aux[:, RNC] = BIAS_RN
    aux[:, BEC] = BIAS_E
    aux[0, WVC:WVC + 16] = 1.0
    aux[0, WVC + 16:WVC + 20] = -1.0

    auxh = np.zeros((128, 8), dtype=np.float16)
    auxh[:, 0] = 1.0
    for l in range(3):
        Co = COUT[l]
        auxh[0:Co, SEL2[l]] = 1.0
        auxh[64:64 + Co, SEL2[l] + 1] = 1.0
    auxh2 = np.zeros((2, 512), dtype=np.float16)
    auxh2[0, 0:128] = 1.0
    for l in range(3):
        Co = COUT[l]
        auxh2[0, BSEL[l]:BSEL[l] + Co] = 1.0
        auxh2[1, BSEL[l] + 64:BSEL[l] + 64 + Co] = 1.0

    # partition-major x blobs: [128, NCH*576] fp8, chunk n at cols n*576,
    # [neigh 512 | center 64] per chunk
    f8 = mybir.dt.np(F8)
    xqs = [np.zeros((128, NCH * 576), dtype=f8) for _ in range(NCORES)]
    xks = [np.zeros((128, NCH * 576), dtype=f8) for _ in range(NCORES)]
    for l in range(4):
        C = CS[l]
        fq = np.asarray(inputs[f"fq{l}"])[:, :, :32, :32].reshape(NCORES, C, 1024)
        fk = np.asarray(inputs[f"fk{l}"])[:, :, :32, :32].reshape(NCORES, C, 1024)
        qn = fq[:, :, nidx[l]].astype(f8)
        qc = fq[:, :, cidx[l]].astype(f8)
        kn = fk[:, :, nidx[l]].astype(f8)
        kc_ = fk[:, :, cidx[l]].astype(f8)
        for b in range(NCORES):
            for k in range(KC[l]):
                n0 = CH[(l, k)] * 576
                rows = min(128, C - k * 128)
                sl = slice(k * 128, k * 128 + rows)
                xqs[b][0:rows, n0:n0 + 512] = qn[b, sl, :]
                xqs[b][0:rows, n0 + 512:n0 + 576] = qc[b, sl, :]
                xks[b][0:rows, n0:n0 + 512] = kn[b, sl, :]
                xks[b][0:rows, n0 + 512:n0 + 576] = kc_[b, sl, :]
    return wts, wts8, aux, auxh, auxh2, xqs, xks


_LAST_RESULT = {}


def kernel(**inputs):
    assert int(inputs.get("start_layer", 0)) == 0
    assert int(inputs.get("end_layer", 4)) == 4
    assert int(inputs.get("num_s", 64)) == 64

    nc = _get_nc()
    wts, wts8, aux, auxh, auxh2, xqs, xks = _host_blobs(inputs)
    in_maps = [
        {"xq": xqs[b], "xk": xks[b], "wts": wts, "wts8": wts8, "aux": aux,
         "auxh": auxh, "auxh2": auxh2}
        for b in range(NCORES)
    ]
    r = run_bass_kernel_spmd(nc, in_maps, core_ids=list(range(NCORES)))
    _LAST_RESULT["r"] = r
    partials = [np.float64(r.results[b]["out"][0, 0]) for b in range(NCORES)]
    loss = np.float32(sum(partials) / (NCORES * S))
    return np.asarray(loss, dtype=np.float32)
